# revision 26
# baseline (speedup 1.0000x reference)
"""Trainium2 Bass kernel for nn_CombinedLoss (chamfer + repulsion + PCA-normal
consistency) on point clouds [8, 2048, 3].

Sharding: data-parallel over batch B=8 across 8 NeuronCores (1 sample/core).

v2 — restructured for the axon tunnel's ~60-80ms/RPC latency (the baseline
spent ~0.95s/call on host-prepped input upload, 6 per-tensor output fetches,
and per-call jit retracing):
  - raw pred/gt uploaded (392KB total); ALL input prep happens on device
    (squared norms, fp32 distance-matmul operand rows, feature rows and
    their bf16 hi/lo transposed layout for the covariance matmul)
  - distance matrices -D via fp32 PE matmuls with K=5 augmented contraction
  - chamfer row/col reductions and the repulsion moment inversion are
    reduced to per-core SCALARS on device
  - per-point 3x3 PCA covariances are centered ON DEVICE and emitted as f16
    [xx,xy,xz,yy,yz,zz] rows in two output tensors (cloud p / cloud g +
    scalars), fetched concurrently (~0.4MB total)
  - the jitted shard_map executable is built once and cached; the donated
    output buffers are recycled from the previous call's outputs
Host: smallest-eigval eigenvectors via a numba scalar port of the fp32
LAPACK-ssyevd sign-convention replica (validated 100% against jax CPU eigh
signs; ~9ms for all 32768 matrices), then the weighted loss.
"""

import numpy as np
from concurrent.futures import ThreadPoolExecutor

try:
    import ml_dtypes

    BF16 = ml_dtypes.bfloat16
except Exception:  # pragma: no cover
    BF16 = None

B, N, DIM = 8, 2048, 3
K_REP = 4
REP_THRESH = np.float32(0.02)
R2 = float(np.float32(REP_THRESH) * np.float32(REP_THRESH))
K_NORM = 16
CD_W, REP_W, NORM_W = 1.0, 0.1, 0.01
NB = N // 128  # 16 row blocks
NEG_BIG = np.float32(-1e30)


# ============================================================================
# Bass device kernel builder
# ============================================================================

def _build_nc(split_waits=True):
    import concourse.bass as bass
    import concourse.mybir as mybir
    from concourse.tile import TileContext

    f32 = mybir.dt.float32
    bf16 = mybir.dt.bfloat16
    Alu = mybir.AluOpType
    Act = mybir.ActivationFunctionType
    Axis = mybir.AxisListType

    nc = bass.Bass()

    # ---- DRAM io (declaration order == jit operand order) ----
    f16 = mybir.dt.float16
    pred = nc.dram_tensor("pred", [N, DIM], f32, kind="ExternalInput")
    gt = nc.dram_tensor("gt", [N, DIM], f32, kind="ExternalInput")
    ident = nc.dram_tensor("ident", [128, 128], bf16, kind="ExternalInput")
    negdiag = nc.dram_tensor("negdiag", [128, 128], bf16, kind="ExternalInput")
    # centered covariance entries [xx,xy,xz,yy,yz,zz] per point, f16;
    # out_g additionally carries the 4 scalars in its last 32 columns
    out_p = nc.dram_tensor("out_p", [6, N], f16, kind="ExternalOutput")
    out_g = nc.dram_tensor("out_g", [6, N + 32], f16, kind="ExternalOutput")

    with TileContext(nc) as tc:
        import contextlib
        ctx = contextlib.ExitStack()
        with ctx:
            persist = ctx.enter_context(tc.tile_pool(name="persist", bufs=1))
            big = ctx.enter_context(tc.tile_pool(name="big", bufs=1))
            scrp = ctx.enter_context(tc.tile_pool(name="scr", bufs=1))
            ndmp = ctx.enter_context(tc.tile_pool(name="ndm", bufs=2))
            wtp = ctx.enter_context(tc.tile_pool(name="wtp", bufs=2))
            psd = ctx.enter_context(tc.tile_pool(name="psd", bufs=2, space="PSUM"))
            psc = ctx.enter_context(tc.tile_pool(name="psc", bufs=1, space="PSUM"))

            # ---- consts ----
            t_ident = persist.tile([128, 128], bf16, tag="ident")
            t_negdiag = persist.tile([128, 128], bf16, tag="ndg")
            nc.sync.dma_start(t_ident[:], ident[:])
            nc.sync.dma_start(t_negdiag[:], negdiag[:])
            t_ones = persist.tile([128, 128], bf16, tag="ones")
            nc.vector.memset(t_ones[:], 1.0)
            t_bias4 = persist.tile([128, 1], f32, tag="bias4")
            t_bias0 = persist.tile([128, 1], f32, tag="bias0")
            nc.vector.memset(t_bias4[:], R2)
            nc.vector.memset(t_bias0[:], 0.0)
            t_ones6 = persist.tile([1, 8], f32, tag="ones6")
            nc.vector.memset(t_ones6[:], 1.0)

            # ---- persistent per-cloud operand tiles ----
            A5 = {}; W5 = {}; F10 = {}; FT = {}
            for cl in ("p", "g"):
                A5[cl] = persist.tile([5, N], f32, tag=f"A5{cl}", name=f"A5{cl}")
                W5[cl] = persist.tile([5, N], f32, tag=f"W5{cl}", name=f"W5{cl}")
                F10[cl] = persist.tile([10, N], f32, tag=f"F10{cl}", name=f"F10{cl}")
                FT[cl] = persist.tile([128, NB * 20], bf16, tag=f"FT{cl}",
                                      name=f"FT{cl}")

            t_rowmax = persist.tile([128, NB, 2], f32, tag="rowmax")
            t_s1 = persist.tile([128, NB], f32, tag="s1")
            t_s2 = persist.tile([128, NB], f32, tag="s2")

            # ================= on-device prep =================
            # A5 = [2x, 2y, 2z, nn, 1] (fp32 matmul lhs rows)
            # W5 = [x, y, z, -1, -nn]  (fp32 matmul rhs rows)
            # F10 = [x2,xy,xz,y2,yz,z2,x,y,z,1] of centered coords
            # FT  = transposed bf16 hi/lo features [128, kb*20 + (0:10 hi|10:20 lo)]
            def prep(src_dram, cl):
                P3 = scrp.tile([3, N], f32, tag="P3")
                nc.sync.dma_start(P3[:], src_dram[:].rearrange("a b -> b a"))
                S3 = scrp.tile([3, N], f32, tag="S3")
                nc.vector.tensor_tensor(S3[:], P3[:], P3[:], Alu.mult)
                r1 = scrp.tile([1, N], f32, tag="r1")
                r2t = scrp.tile([1, N], f32, tag="r2t")
                nc.sync.dma_start(r1[:], S3[1:2, :])
                nc.sync.dma_start(r2t[:], S3[2:3, :])
                nn = scrp.tile([1, N], f32, tag="nn")
                nc.vector.tensor_tensor(nn[:], S3[0:1, :], r1[:], Alu.add)
                nc.vector.tensor_tensor(nn[:], nn[:], r2t[:], Alu.add)
                # engine ops may only start at partitions {0,32,64,96}: memset
                # the whole tile for the constant rows, DMA the odd-row writes
                a5, w5 = A5[cl], W5[cl]
                nc.vector.memset(a5[:], 1.0)
                nc.scalar.activation(a5[0:3, :], P3[:], Act.Copy, scale=2.0)
                nc.sync.dma_start(a5[3:4, :], nn[:])
                nc.vector.memset(w5[:], -1.0)
                nc.vector.tensor_copy(w5[0:3, :], P3[:])
                nnn = scrp.tile([1, N], f32, tag="nnn")
                nc.scalar.activation(nnn[:], nn[:], Act.Copy, scale=-1.0)
                nc.sync.dma_start(w5[4:5, :], nnn[:])
                # centered features
                C3 = scrp.tile([3, N], f32, tag="C3")
                nc.vector.tensor_scalar_add(C3[:], P3[:], -0.5)
                A6 = scrp.tile([6, N], f32, tag="A6")
                B6 = scrp.tile([6, N], f32, tag="B6")
                # A6 rows = [c0,c0,c0,c1,c1,c2]; B6 rows = [c0,c1,c2,c1,c2,c2]
                nc.vector.tensor_copy(A6[0:1, :], C3[0:1, :])
                nc.sync.dma_start(A6[1:2, :], C3[0:1, :])
                nc.sync.dma_start(A6[2:3, :], C3[0:1, :])
                nc.sync.dma_start(A6[3:4, :], C3[1:2, :])
                nc.sync.dma_start(A6[4:5, :], C3[1:2, :])
                nc.sync.dma_start(A6[5:6, :], C3[2:3, :])
                nc.vector.tensor_copy(B6[0:3, :], C3[:])
                nc.sync.dma_start(B6[3:5, :], C3[1:3, :])
                nc.sync.dma_start(B6[5:6, :], C3[2:3, :])
                f10 = F10[cl]
                nc.vector.memset(f10[:], 1.0)
                nc.vector.tensor_tensor(f10[0:6, :], A6[:], B6[:], Alu.mult)
                nc.sync.dma_start(f10[6:9, :], C3[:])
                # bf16 hi/lo split of features
                hi10 = scrp.tile([10, N], bf16, tag="hi10")
                hif = scrp.tile([10, N], f32, tag="hif")
                lo10f = scrp.tile([10, N], f32, tag="lo10f")
                lo10 = scrp.tile([10, N], bf16, tag="lo10")
                nc.scalar.activation(hi10[:], f10[:], Act.Copy)
                nc.scalar.activation(hif[:], hi10[:], Act.Copy)
                nc.vector.tensor_tensor(lo10f[:], f10[:], hif[:], Alu.subtract)
                nc.scalar.activation(lo10[:], lo10f[:], Act.Copy)
                # transpose [10, 128]-chunks -> FT[:, kb*20 + 0:10 / 10:20]
                ftt = FT[cl]
                for b in range(NB):
                    csl = slice(b * 128, (b + 1) * 128)
                    pst = psd.tile([128, 16], bf16, tag="dps")
                    nc.tensor.transpose(pst[:, 0:10], hi10[:, csl],
                                        t_ident[0:10, 0:10])
                    nc.scalar.activation(ftt[:, b * 20:b * 20 + 10], pst[:, 0:10],
                                         Act.Copy)
                    pst2 = psd.tile([128, 16], bf16, tag="dps")
                    nc.tensor.transpose(pst2[:, 0:10], lo10[:, csl],
                                        t_ident[0:10, 0:10])
                    nc.scalar.activation(ftt[:, b * 20 + 10:b * 20 + 20],
                                         pst2[:, 0:10], Act.Copy)

            prep(pred, "p")
            prep(gt, "g")

            # fp32 distance matmul: psum[128, 1024] = -D block (row block b,
            # column half h) between clouds (a5 lhs, w5 rhs)
            def build_half(a5, w5, b, h, ps):
                for j in range(2):
                    nc.tensor.matmul(
                        ps[:, j * 512:(j + 1) * 512],
                        a5[:, b * 128:(b + 1) * 128],
                        w5[:, h * 1024 + j * 512:h * 1024 + (j + 1) * 512],
                        start=True, stop=True,
                    )

            # ================= phase 1: chamfer on -Dpg =================
            t_colacc = big.tile([128, N], f32, tag="bigA")
            t_colred = big.tile([128, N], f32, tag="bigB")
            for b in range(NB):
                for h in range(2):
                    ps = psd.tile([128, 1024], f32, tag="dps")
                    build_half(A5["p"], W5["g"], b, h, ps)
                    nc.vector.tensor_reduce(t_rowmax[:, b, h:h + 1],
                                            ps[:], Axis.X, Alu.max)
                    cslice = slice(h * 1024, (h + 1) * 1024)
                    if b == 0:
                        nc.vector.tensor_copy(t_colacc[:, cslice], ps[:])
                    else:
                        nc.vector.tensor_tensor(t_colacc[:, cslice],
                                                t_colacc[:, cslice], ps[:], Alu.max)
            # partition-tree max 128 -> 1 (DMA crosses partitions, DVE cannot)
            for h in [64, 32, 16, 8, 4, 2, 1]:
                nc.sync.dma_start(t_colred[0:h, :], t_colacc[h:2 * h, :])
                nc.vector.tensor_tensor(t_colacc[0:h, :], t_colacc[0:h, :],
                                        t_colred[0:h, :], Alu.max)
            # chamfer scalars: sum of per-row maxes + sum of col maxes (of -D)
            t_cdcol = persist.tile([1, 1], f32, tag="cdcol")
            nc.vector.tensor_reduce(t_cdcol[:], t_colacc[0:1, :], Axis.X, Alu.add)
            t_rowfull = scrp.tile([128, NB], f32, tag="rowfull")
            nc.vector.tensor_reduce(t_rowfull[:], t_rowmax[:], Axis.X, Alu.max)
            t_cdrow = persist.tile([128, 1], f32, tag="cdrow")
            nc.vector.tensor_reduce(t_cdrow[:], t_rowfull[:], Axis.X, Alu.add)

            # ================= phases 2-4 for pp and gg =================
            def normals_phase(cl, out_dram, do_rep):
                a5, w5, ftt, f10 = A5[cl], W5[cl], FT[cl], F10[cl]

                def build_ndm(b):
                    ndm = ndmp.tile([128, N], bf16, tag="ndm", name=f"ndm{cl}{b}")
                    for h in range(2):
                        ps = psd.tile([128, 1024], f32, tag="dps")
                        build_half(a5, w5, b, h, ps)
                        nc.scalar.activation(ndm[:, h * 1024:(h + 1) * 1024],
                                             ps[:], Act.Copy)
                    nc.vector.tensor_tensor(
                        ndm[:, b * 128:(b + 1) * 128],
                        ndm[:, b * 128:(b + 1) * 128],
                        t_negdiag[:], Alu.add)
                    return ndm

                # pass 1: repulsion moments + 16-NN radius (tau) per row
                t_tau = scrp.tile([128, NB], f32, tag="tau")
                for b in range(NB):
                    ndm = build_ndm(b)
                    if do_rep:
                        scr = scrp.tile([128, N], bf16, tag="repscr")
                        scr2 = scrp.tile([128, N], bf16, tag="repscr2")
                        nc.scalar.activation(scr[:], ndm[:], Act.Relu,
                                             bias=t_bias4[:],
                                             accum_out=t_s1[:, b:b + 1])
                        nc.scalar.activation(scr2[:], scr[:], Act.Square,
                                             bias=t_bias0[:],
                                             accum_out=t_s2[:, b:b + 1])
                    t1 = scrp.tile([128, 1024], bf16, tag="tree1")
                    At = scrp.tile([128, 512], bf16, tag="treeA")
                    At2 = scrp.tile([128, 512], bf16, tag="treeA2")
                    m8a = scrp.tile([128, 8], bf16, tag="m8a")
                    m8b = scrp.tile([128, 8], bf16, tag="m8b")
                    nc.vector.tensor_tensor(t1[:], ndm[:, 0:1024],
                                            ndm[:, 1024:2048], Alu.max)
                    nc.vector.tensor_tensor(At[:], t1[:, 0:512],
                                            t1[:, 512:1024], Alu.max)
                    nc.vector.max(m8a[:], At[:])
                    nc.vector.match_replace(At2[:], m8a[:], At[:], float(NEG_BIG))
                    nc.vector.max(m8b[:], At2[:])
                    nc.vector.tensor_copy(t_tau[:, b:b + 1], m8b[:, 6:7])

                # tau broadcast: per-row tau -> [1, N] row -> PE ones-matmul
                # broadcast across partitions; mask compare is then direct on
                # the SYMMETRIC ndm blocks: wt[j, i] = (ndm[j, i] >= tau_i)
                t_taub = scrp.tile([128, 128], bf16, tag="taub")
                nc.vector.memset(t_taub[:], 0.0)
                nc.vector.tensor_copy(t_taub[:, 0:NB], t_tau[:])
                ps_tt = psd.tile([128, 128], bf16, tag="dps")
                nc.tensor.transpose(ps_tt[:], t_taub[:], t_ident[:])
                t_tt = scrp.tile([NB, 128], bf16, tag="tts")
                nc.scalar.activation(t_tt[:], ps_tt[0:NB, :], Act.Copy)
                t_tauT = scrp.tile([128, N], bf16, tag="tauT")
                nc.vector.memset(t_tauT[:], 0.0)
                nc.sync.dma_start(t_tauT[0:1, :], t_tt[:])
                t_taubc = scrp.tile([128, N], bf16, tag="taubc")
                for h in range(2):
                    ps_tau = psd.tile([128, 1024], f32, tag="dps")
                    for bb in range(8):
                        c0 = h * 1024 + bb * 128
                        nc.tensor.matmul(ps_tau[:, bb * 128:(bb + 1) * 128],
                                         t_ones[:],
                                         t_tauT[:, c0:c0 + 128],
                                         start=True, stop=True)
                    nc.scalar.activation(t_taubc[:, h * 1024:(h + 1) * 1024],
                                         ps_tau[:], Act.Copy)

                # pass 2: rebuild -D per block, mask, accumulate covariance
                # moments cps[10, N] over kb (hi+lo)
                cps = psc.tile([10, N], f32, tag="cps")
                for kb in range(NB):
                    ndm = build_ndm(kb)
                    wt = wtp.tile([128, N], bf16, tag="wt", name=f"wt{cl}{kb}")
                    nc.vector.tensor_tensor(wt[:], ndm[:], t_taubc[:], Alu.is_ge)
                    for j in range(4):
                        cols = slice(j * 512, (j + 1) * 512)
                        for half in range(2):
                            nc.tensor.matmul(
                                cps[:, cols],
                                ftt[:, kb * 20 + half * 10:kb * 20 + (half + 1) * 10],
                                wt[:, cols],
                                start=(kb == 0 and half == 0),
                                stop=(kb == NB - 1 and half == 1))
                # self add, then center on device:
                #   covc[ab] = M2[ab]/cnt - (s[a]/cnt)*(s[b]/cnt)   (f16 out)
                covsb = big.tile([10, N], f32, tag="bigA", name=f"covsb{cl}")
                nc.vector.tensor_tensor(covsb[:], cps[:], f10[:], Alu.add)
                rr = scrp.tile([1, N], f32, tag="r1")
                nc.sync.dma_start(rr[:], covsb[9:10, :])
                rcp = scrp.tile([1, N], f32, tag="r2t")
                nc.vector.reciprocal(rcp[:], rr[:])
                mus = scrp.tile([3, N], f32, tag="S3")
                nc.sync.dma_start(mus[:], covsb[6:9, :])
                psB3 = psc.tile([3, N], f32, tag="cps", name=f"psB3{cl}")
                for j in range(4):
                    cj = slice(j * 512, (j + 1) * 512)
                    nc.tensor.matmul(psB3[:, cj], t_ones6[0:1, 0:3], rcp[:, cj],
                                     start=True, stop=True)
                mu3 = scrp.tile([3, N], f32, tag="C3")
                nc.vector.tensor_tensor(mu3[:], mus[:], psB3[:], Alu.mult)
                A6m = scrp.tile([6, N], f32, tag="A6")
                B6m = scrp.tile([6, N], f32, tag="B6")
                nc.vector.tensor_copy(A6m[0:1, :], mu3[0:1, :])
                nc.sync.dma_start(A6m[1:2, :], mu3[0:1, :])
                nc.sync.dma_start(A6m[2:3, :], mu3[0:1, :])
                nc.sync.dma_start(A6m[3:4, :], mu3[1:2, :])
                nc.sync.dma_start(A6m[4:5, :], mu3[1:2, :])
                nc.sync.dma_start(A6m[5:6, :], mu3[2:3, :])
                nc.vector.tensor_copy(B6m[0:3, :], mu3[:])
                nc.sync.dma_start(B6m[3:5, :], mu3[1:3, :])
                nc.sync.dma_start(B6m[5:6, :], mu3[2:3, :])
                P6 = scrp.tile([6, N], f32, tag="lo10f")
                nc.vector.tensor_tensor(P6[:], A6m[:], B6m[:], Alu.mult)
                psB6 = psc.tile([6, N], f32, tag="cps", name=f"psB6{cl}")
                for j in range(4):
                    cj = slice(j * 512, (j + 1) * 512)
                    nc.tensor.matmul(psB6[:, cj], t_ones6[0:1, 0:6], rcp[:, cj],
                                     start=True, stop=True)
                M2r = scrp.tile([6, N], f32, tag="hif")
                nc.vector.tensor_tensor(M2r[:], covsb[0:6, :], psB6[:], Alu.mult)
                covc = scrp.tile([6, N], f16, tag="hi10")
                nc.vector.tensor_tensor(covc[:], M2r[:], P6[:], Alu.subtract)
                nc.sync.dma_start(out_dram[:, 0:N], covc[:])

            normals_phase("p", out_p, do_rep=True)

            # ---- repulsion moment inversion -> per-row contribution ----
            # a,b = (s1 +- sqrt(2*s2 - s1^2))/2; d=sqrt(r2-v); contrib =
            # relu(0.02-da)+relu(0.02-db), gated by s1>0
            sh = [128, NB]
            t_t1 = scrp.tile(sh, f32, tag="rp1")
            t_t2 = scrp.tile(sh, f32, tag="rp2")
            t_sq = scrp.tile(sh, f32, tag="rp3")
            t_va = scrp.tile(sh, f32, tag="rp4")
            t_vb = scrp.tile(sh, f32, tag="rp5")
            t_ca = scrp.tile(sh, f32, tag="rp6")
            t_cb = scrp.tile(sh, f32, tag="rp7")
            t_msk = scrp.tile(sh, f32, tag="rp8")
            Alu_ = Alu
            nc.vector.tensor_tensor(t_t1[:], t_s1[:], t_s1[:], Alu_.mult)
            nc.vector.tensor_scalar(t_t2[:], t_s2[:], 2.0, None, Alu_.mult)
            nc.vector.tensor_tensor(t_t2[:], t_t2[:], t_t1[:], Alu_.subtract)
            nc.vector.tensor_scalar_max(t_t2[:], t_t2[:], 0.0)
            nc.scalar.activation(t_sq[:], t_t2[:], Act.Sqrt)
            nc.vector.tensor_tensor(t_va[:], t_s1[:], t_sq[:], Alu_.add)
            nc.vector.tensor_scalar(t_va[:], t_va[:], 0.5, R2, Alu_.mult, Alu_.min)
            nc.vector.tensor_tensor(t_vb[:], t_s1[:], t_sq[:], Alu_.subtract)
            nc.vector.tensor_scalar(t_vb[:], t_vb[:], 0.5, 0.0, Alu_.mult, Alu_.max)
            # da = sqrt(max(r2 - va, 1e-12)); contrib_a = max(0.02 - da, 0)
            for tv, tc_ in ((t_va, t_ca), (t_vb, t_cb)):
                nc.vector.tensor_scalar(tv[:], tv[:], -1.0, R2, Alu_.mult, Alu_.add)
                nc.vector.tensor_scalar_max(tv[:], tv[:], 1e-12)
                nc.scalar.activation(tv[:], tv[:], Act.Sqrt)
                nc.vector.tensor_scalar(tc_[:], tv[:], -1.0, float(REP_THRESH),
                                        Alu_.mult, Alu_.add)
                nc.vector.tensor_scalar_max(tc_[:], tc_[:], 0.0)
            nc.vector.tensor_scalar(t_msk[:], t_s1[:], 0.0, None, Alu_.is_gt)
            nc.vector.tensor_tensor(t_ca[:], t_ca[:], t_cb[:], Alu_.add)
            nc.vector.tensor_tensor(t_ca[:], t_ca[:], t_msk[:], Alu_.mult)
            t_reprow = persist.tile([128, 1], f32, tag="reprow")
            nc.vector.tensor_reduce(t_reprow[:], t_ca[:], Axis.X, Alu_.add)

            # ---- partition-sum [cd_row, rep] via DMA tree; pack scalars ----
            t_P2 = scrp.tile([128, 2], f32, tag="P2")
            t_P2s = scrp.tile([64, 2], f32, tag="P2s")
            nc.vector.tensor_copy(t_P2[:, 0:1], t_cdrow[:])
            nc.vector.tensor_copy(t_P2[:, 1:2], t_reprow[:])
            for h in [64, 32, 16, 8, 4, 2, 1]:
                nc.sync.dma_start(t_P2s[0:h, :], t_P2[h:2 * h, :])
                nc.vector.tensor_tensor(t_P2[0:h, :], t_P2[0:h, :],
                                        t_P2s[0:h, :], Alu.add)
            t_z6 = scrp.tile([6, 32], f16, tag="z6")
            nc.vector.memset(t_z6[:], 0.0)
            nc.vector.tensor_copy(t_z6[0:1, 0:2], t_P2[0:1, :])
            nc.vector.tensor_copy(t_z6[0:1, 2:3], t_cdcol[:])
            nc.sync.dma_start(out_g[:, N:N + 32], t_z6[:])

            normals_phase("g", out_g, do_rep=False)

    if split_waits:
        _split_excess_waits(nc, mybir)
    return nc


def _split_excess_waits(nc, mybir, max_w=1, max_u=1):
    """This toolchain's walrus accepts at most 1 sync wait and 1 update per
    instruction. Move excess waits onto same-engine prefix NoOps (the engine
    is in-order, so waiting earlier is equivalent) and excess updates onto
    suffix NoOps (signalling marginally later is safe)."""
    n = 0
    for func in nc.m.functions:
        for block in func.blocks:
            lst = block.instructions
            new = []
            for inst in lst:
                si = inst.sync_info
                ow = list(si.on_wait) if (si and si.on_wait) else []
                if len(ow) > max_w:
                    extra, keep = ow[:-max_w], ow[-max_w:]
                    for k in range(0, len(extra), max_w):
                        nop = mybir.InstNoOp(name=f"I-wsplit-{n}"); n += 1
                        nop.engine = inst.engine
                        nop.sync_info = mybir.SyncInfo(
                            on_wait=extra[k:k + max_w], on_update=[])
                        new.append(nop)
                    si.on_wait = keep
                new.append(inst)
                ou = list(si.on_update) if (si and si.on_update) else []
                if len(ou) > max_u:
                    keep_u, extra_u = ou[:max_u], ou[max_u:]
                    si.on_update = keep_u
                    for k in range(0, len(extra_u), max_u):
                        nop = mybir.InstNoOp(name=f"I-usplit-{n}"); n += 1
                        nop.engine = inst.engine
                        nop.sync_info = mybir.SyncInfo(
                            on_wait=[], on_update=extra_u[k:k + max_u])
                        new.append(nop)
            lst[:] = new
    return n


_NC_CACHE = None


def _get_nc():
    global _NC_CACHE
    if _NC_CACHE is None:
        _NC_CACHE = _build_nc()
    return _NC_CACHE


def _consts_np():
    negdiag = np.zeros((128, 128), dtype=BF16)
    np.fill_diagonal(negdiag, BF16(NEG_BIG))
    ident = np.zeros((128, 128), dtype=BF16)
    np.fill_diagonal(ident, BF16(1.0))
    return ident, negdiag


# ============================================================================
# Cached jit runner (replicates bass2jax.run_bass_via_pjrt, but the jitted
# executable, mesh, and const device buffers are built ONCE; the donated
# output buffer is recycled from the previous call's output)
# ============================================================================

class _Runner:
    def __init__(self):
        import jax
        from jax.sharding import Mesh, PartitionSpec, NamedSharding
        from jax.experimental.shard_map import shard_map
        from concourse import bass2jax
        import concourse.mybir as mybir

        self.jax = jax
        nc = _get_nc()
        bass2jax.install_neuronx_cc_hook()

        partition_name = (nc.partition_id_tensor.name
                          if nc.partition_id_tensor else None)
        in_names, out_names, out_avals, zero_outs = [], [], [], []
        for alloc in nc.m.functions[0].allocations:
            if not isinstance(alloc, mybir.MemoryLocationSet):
                continue
            name = alloc.memorylocations[0].name
            if alloc.kind == "ExternalInput":
                if name != partition_name:
                    in_names.append(name)
            elif alloc.kind == "ExternalOutput":
                shape = tuple(alloc.tensor_shape)
                dtype = mybir.dt.np(alloc.dtype)
                out_names.append(name)
                out_avals.append(jax.core.ShapedArray(shape, dtype))
                zero_outs.append((shape, dtype))
        assert in_names == ["pred", "gt", "ident", "negdiag"], in_names
        assert out_names == ["out_p", "out_g"], out_names
        n_params = len(in_names)
        n_outs = len(out_names)
        all_names = in_names + out_names
        if partition_name is not None:
            all_names.append(partition_name)
        self.zero_outs = zero_outs

        def _body(*args):
            operands = list(args)
            if partition_name is not None:
                operands.append(bass2jax.partition_id_tensor())
            outs = bass2jax._bass_exec_p.bind(
                *operands,
                out_avals=tuple(out_avals),
                in_names=tuple(all_names),
                out_names=tuple(out_names),
                lowering_input_output_aliases=(),
                sim_require_finite=True,
                sim_require_nnan=True,
                nc=nc,
            )
            return tuple(outs)

        devices = jax.devices()[:B]
        assert len(devices) == B, f"need {B} devices, have {len(jax.devices())}"
        mesh = Mesh(np.asarray(devices), ("core",))
        pspec = PartitionSpec("core")
        self._fn = jax.jit(
            shard_map(_body, mesh=mesh,
                      in_specs=(pspec,) * (n_params + n_outs),
                      out_specs=(pspec,) * n_outs,
                      check_rep=False),
            donate_argnums=tuple(range(n_params, n_params + n_outs)),
            keep_unused=True,
        )
        ident, negdiag = _consts_np()
        sh = NamedSharding(mesh, pspec)
        self._ident = jax.device_put(np.tile(ident, (B, 1)), sh)
        self._negdiag = jax.device_put(np.tile(negdiag, (B, 1)), sh)
        self._donate = None  # recycled output buffers

    def run(self, pred, gt):
        """pred, gt: [B, N, 3] f32 -> (fut_p, fut_g) resolving to host
        np.ndarrays [B*6, N] / [B*6, N+32] f16."""
        zeros = self._donate
        if zeros is None:
            zeros = [np.zeros((B * s[0],) + s[1:], d)
                     for s, d in self.zero_outs]
        out_p, out_g = self._fn(pred.reshape(B * N, DIM),
                                gt.reshape(B * N, DIM),
                                self._ident, self._negdiag, *zeros)
        # queue both host transfers EAGERLY: an np.asarray issued after the
        # ready notification pays a full extra tunnel round trip (~100ms);
        # copy_to_host_async rides the execute pipeline instead, and cloud
        # g's transfer proceeds in background while the host runs cloud p's
        # eigensolve
        try:
            out_p.copy_to_host_async()
            out_g.copy_to_host_async()
        except Exception:
            pass
        # the kernel writes every element of both outputs, so last call's
        # outputs can be donated as the next call's output buffers
        self._donate = [out_p, out_g]
        return out_p, out_g


_RUNNER = None


def _get_runner():
    global _RUNNER
    if _RUNNER is None:
        _RUNNER = _Runner()
    return _RUNNER


# ============================================================================
# Host combine
# ============================================================================

# ----------------------------------------------------------------------------
# LAPACK ssyevd 3x3 sign-convention replication (fp32), numba scalar port of
# the vectorized replica validated 100% against jax/scipy CPU eigh signs.
# Falls back to np.linalg.eigh (99.35% sign agreement) without numba.
# ----------------------------------------------------------------------------
try:
    from numba import njit as _njit
    _HAVE_NUMBA = True
except Exception:  # pragma: no cover
    _HAVE_NUMBA = False

if _HAVE_NUMBA:
    _F = np.float32
    _EPS = _F(2.0) ** _F(-24)
    _EPS2 = _F(_EPS * _EPS)
    _SAFMIN = _F(1.1754943508222875e-38)
    _ONE = _F(1.0)
    _TWO = _F(2.0)
    _HALF = _F(0.5)
    _ZERO = _F(0.0)

    @_njit(cache=True, fastmath=False)
    def _fsign(a, b):
        return np.abs(a) if b >= _ZERO else -np.abs(a)

    @_njit(cache=True, fastmath=False)
    def _slapy2(x, y):
        ax = np.abs(x); ay = np.abs(y)
        w = max(ax, ay); z = min(ax, ay)
        if z == _ZERO:
            return w
        r = z / w
        return w * np.sqrt(_ONE + r * r)

    @_njit(cache=True, fastmath=False)
    def _slartg(f, g):
        if g == _ZERO:
            return _ONE, _ZERO, f
        if f == _ZERO:
            return _ZERO, _fsign(_ONE, g), np.abs(g)
        d = np.sqrt(f * f + g * g)
        cs = np.abs(f) / d
        r = _fsign(d, f)
        sn = g / r
        return cs, sn, r

    @_njit(cache=True, fastmath=False)
    def _slaev2(a, b, c):
        sm = a + c
        df = a - c
        adf = np.abs(df)
        tb = b + b
        ab_ = np.abs(tb)
        if np.abs(a) > np.abs(c):
            acmx = a; acmn = c
        else:
            acmx = c; acmn = a
        if adf > ab_:
            r_ = ab_ / adf
            rt = adf * np.sqrt(_ONE + r_ * r_)
        elif adf < ab_:
            r_ = adf / ab_
            rt = ab_ * np.sqrt(_ONE + r_ * r_)
        else:
            rt = ab_ * np.sqrt(_TWO)
        if sm < _ZERO:
            rt1 = _HALF * (sm - rt)
            sgn1 = -_ONE
            rt2 = (acmx / rt1) * acmn - (b / rt1) * b
        elif sm > _ZERO:
            rt1 = _HALF * (sm + rt)
            sgn1 = _ONE
            rt2 = (acmx / rt1) * acmn - (b / rt1) * b
        else:
            rt1 = _HALF * rt
            sgn1 = _ONE
            rt2 = -_HALF * rt
        if df >= _ZERO:
            cs = df + rt
            sgn2 = _ONE
        else:
            cs = df - rt
            sgn2 = -_ONE
        acs = np.abs(cs)
        if acs > ab_:
            ct = -tb / cs
            sn1 = _ONE / np.sqrt(_ONE + ct * ct)
            cs1 = ct * sn1
        else:
            if ab_ == _ZERO:
                cs1 = _ONE
                sn1 = _ZERO
            else:
                tn = -cs / tb
                cs1 = _ONE / np.sqrt(_ONE + tn * tn)
                sn1 = tn * cs1
        if sgn1 == sgn2:
            t = cs1
            cs1 = -sn1
            sn1 = t
        return rt1, rt2, cs1, sn1

    @_njit(cache=True, fastmath=False)
    def _rot(Z, ca, cb, c, s):
        for i in range(3):
            temp = Z[i, cb]
            Z[i, cb] = c * temp - s * Z[i, ca]
            Z[i, ca] = s * temp + c * Z[i, ca]

    @_njit(cache=True, fastmath=False)
    def _eigh3_batch(cv, out):
        # cv: [Bc, 6, N] f32 rows (xx, xy, xz, yy, yz, zz); out: [Bc*N, 3]
        Z = np.empty((3, 3), np.float32)
        n_pts = cv.shape[2]
        for idx in range(cv.shape[0] * n_pts):
            bb = idx // n_pts
            nn_ = idx - bb * n_pts
            a00 = cv[bb, 0, nn_]; a10 = cv[bb, 1, nn_]; a20 = cv[bb, 2, nn_]
            a11 = cv[bb, 3, nn_]; a21 = cv[bb, 4, nn_]; a22 = cv[bb, 5, nn_]
            # ssytd2 lower
            xnorm = np.abs(a20)
            alpha = a10
            beta = -_fsign(_slapy2(alpha, xnorm), alpha)
            refl = xnorm != _ZERO
            if refl:
                tau1 = (beta - alpha) / beta
                v2 = a20 / (alpha - beta)
                w1 = tau1 * a11 + tau1 * (a21 * v2)
                w2 = tau1 * a21 + (tau1 * v2) * a22
                alp = -_HALF * tau1 * (w1 + w2 * v2)
                w1 = w1 + alp
                w2 = w2 + alp * v2
                d0 = a00
                d1 = a11 - (w1 + w1)
                d2 = a22 - ((v2 * w2) + (v2 * w2))
                e0 = beta
                e1 = a21 - (v2 * w1 + w2)
            else:
                tau1 = _ZERO
                v2 = _ZERO
                d0 = a00; d1 = a11; d2 = a22
                e0 = a10; e1 = a21
            for i in range(3):
                for j in range(3):
                    Z[i, j] = _ONE if i == j else _ZERO
            s0 = np.abs(e0) <= (np.sqrt(np.abs(d0)) * np.sqrt(np.abs(d1))) * _EPS
            s1m = np.abs(e1) <= (np.sqrt(np.abs(d1)) * np.sqrt(np.abs(d2))) * _EPS
            if s0:
                e0 = _ZERO
            if s1m:
                e1 = _ZERO
            if s0 and not s1m:
                tst = e1 * e1
                thr = (_EPS2 * np.abs(d1)) * np.abs(d2) + _SAFMIN
                if tst > thr:
                    rt1, rt2, c, s = _slaev2(d1, e1, d2)
                    _rot(Z, 1, 2, c, s)
                    d1 = rt1; d2 = rt2
                e1 = _ZERO
            elif (not s0) and s1m:
                tst = e0 * e0
                thr = (_EPS2 * np.abs(d0)) * np.abs(d1) + _SAFMIN
                if tst > thr:
                    rt1, rt2, c, s = _slaev2(d0, e0, d1)
                    _rot(Z, 0, 1, c, s)
                    d0 = rt1; d1 = rt2
                e0 = _ZERO
            elif (not s0) and (not s1m):
                if np.abs(d2) < np.abs(d0):
                    # QR variant
                    l = 2
                    for _it in range(40):
                        if l <= -1:
                            break
                        if l == 2:
                            m2s = e1 * e1 <= (_EPS2 * np.abs(d2)) * np.abs(d1) + _SAFMIN
                            m1s = e0 * e0 <= (_EPS2 * np.abs(d1)) * np.abs(d0) + _SAFMIN
                            if m2s:
                                e1 = _ZERO
                                l = 1
                            elif m1s:
                                e0 = _ZERO
                                rt1, rt2, c, s = _slaev2(d1, e1, d2)
                                _rot(Z, 1, 2, c, s)
                                d1 = rt1; d2 = rt2
                                e1 = _ZERO
                                l = 0
                            else:
                                P = d2
                                G = (d1 - P) / (_TWO * e1)
                                R = _slapy2(G, _ONE)
                                G = d0 - P + (e1 / (G + _fsign(R, G)))
                                Fv = e0
                                Bv = e0
                                C, S, R = _slartg(G, Fv)
                                G2 = d0
                                R = (d1 - G2) * S + (_TWO * C) * Bv
                                Pv = S * R
                                d0n = G2 + Pv
                                G = C * R - Bv
                                c0 = C; s0_ = S
                                Fv = S * e1
                                Bv = C * e1
                                C, S, R = _slartg(G, Fv)
                                e0n = R
                                G2 = d1 - Pv
                                R = (d2 - G2) * S + (_TWO * C) * Bv
                                Pv2 = S * R
                                d1n = G2 + Pv2
                                G = C * R - Bv
                                c1 = C; s1_ = S
                                _rot(Z, 0, 1, c0, s0_)
                                _rot(Z, 1, 2, c1, s1_)
                                d0 = d0n; d1 = d1n; d2 = d2 - Pv2
                                e0 = e0n; e1 = G
                        elif l == 1:
                            ms = e0 * e0 <= (_EPS2 * np.abs(d1)) * np.abs(d0) + _SAFMIN
                            if ms:
                                e0 = _ZERO
                                l = 0
                            else:
                                rt1, rt2, c, s = _slaev2(d0, e0, d1)
                                _rot(Z, 0, 1, c, s)
                                d0 = rt1; d1 = rt2
                                e0 = _ZERO
                                l = -1
                        else:  # l == 0
                            l = -1
                else:
                    # QL variant
                    l = 0
                    for _it in range(40):
                        if l >= 3:
                            break
                        if l == 0:
                            m0s = e0 * e0 <= (_EPS2 * np.abs(d0)) * np.abs(d1) + _SAFMIN
                            m1s = e1 * e1 <= (_EPS2 * np.abs(d1)) * np.abs(d2) + _SAFMIN
                            if m0s:
                                e0 = _ZERO
                                l = 1
                            elif m1s:
                                e1 = _ZERO
                                rt1, rt2, c, s = _slaev2(d0, e0, d1)
                                _rot(Z, 0, 1, c, s)
                                d0 = rt1; d1 = rt2
                                e0 = _ZERO
                                l = 2
                            else:
                                P = d0
                                G = (d1 - P) / (_TWO * e0)
                                R = _slapy2(G, _ONE)
                                G = d2 - P + (e0 / (G + _fsign(R, G)))
                                Fv = e1
                                Bv = e1
                                C, S, R = _slartg(G, Fv)
                                G2 = d2
                                R = (d1 - G2) * S + (_TWO * C) * Bv
                                Pv = S * R
                                d2n = G2 + Pv
                                G = C * R - Bv
                                c1 = C; s1_ = -S
                                Fv = S * e0
                                Bv = C * e0
                                C, S, R = _slartg(G, Fv)
                                e1n = R
                                G2 = d1 - Pv
                                R = (d0 - G2) * S + (_TWO * C) * Bv
                                Pv2 = S * R
                                d1n = G2 + Pv2
                                G = C * R - Bv
                                c0 = C; s0_ = -S
                                _rot(Z, 1, 2, c1, s1_)
                                _rot(Z, 0, 1, c0, s0_)
                                d2 = d2n; d1 = d1n; d0 = d0 - Pv2
                                e1 = e1n; e0 = G
                        elif l == 1:
                            ms = e1 * e1 <= (_EPS2 * np.abs(d1)) * np.abs(d2) + _SAFMIN
                            if ms:
                                e1 = _ZERO
                                l = 2
                            else:
                                rt1, rt2, c, s = _slaev2(d1, e1, d2)
                                _rot(Z, 1, 2, c, s)
                                d1 = rt1; d2 = rt2
                                e1 = _ZERO
                                l = 3
                        else:  # l == 2
                            l = 3
            # sort eigenvalues ascending, swapping Z columns (ssteqr tail)
            D0 = d0; D1 = d1; D2 = d2
            for i in range(2):
                if i == 0:
                    k = 0; P = D0
                    if D1 < P:
                        k = 1; P = D1
                    if D2 < P:
                        k = 2; P = D2
                    if k != 0:
                        if k == 1:
                            D1 = D0
                        else:
                            D2 = D0
                        D0 = P
                        for r_i in range(3):
                            t = Z[r_i, 0]; Z[r_i, 0] = Z[r_i, k]; Z[r_i, k] = t
                else:
                    if D2 < D1:
                        t2 = D1; D1 = D2; D2 = t2
                        for r_i in range(3):
                            t = Z[r_i, 1]; Z[r_i, 1] = Z[r_i, 2]; Z[r_i, 2] = t
            # back-transform the householder (sorm2r)
            if refl:
                for col in range(3):
                    w = Z[1, col] + v2 * Z[2, col]
                    Z[1, col] = Z[1, col] - tau1 * w
                    Z[2, col] = Z[2, col] - (tau1 * v2) * w
            out[idx, 0] = Z[0, 0]
            out[idx, 1] = Z[1, 0]
            out[idx, 2] = Z[2, 0]


def _normals_from_covc(cv):
    """cv: [B, 6, N] centered covariance rows [xx,xy,xz,yy,yz,zz] (f16) ->
    [B*N, 3] smallest-eigval eigenvectors with ssyevd sign convention."""
    f32 = np.float32
    cv32 = cv.astype(f32)
    if _HAVE_NUMBA:
        out = np.empty((cv32.shape[0] * cv32.shape[2], 3), f32)
        _eigh3_batch(cv32, out)
        return out
    flat = np.ascontiguousarray(cv32.transpose(0, 2, 1).reshape(-1, 6))
    cov = np.empty((flat.shape[0], 3, 3), dtype=f32)
    cov[:, 0, 0] = flat[:, 0]
    cov[:, 0, 1] = cov[:, 1, 0] = flat[:, 1]
    cov[:, 0, 2] = cov[:, 2, 0] = flat[:, 2]
    cov[:, 1, 1] = flat[:, 3]
    cov[:, 1, 2] = cov[:, 2, 1] = flat[:, 4]
    cov[:, 2, 2] = flat[:, 5]
    return np.linalg.eigh(cov)[1][:, :, 0]


def _host_combine(out_p, out_g):
    """out_p/out_g: device outputs [B*6, N] / [B*6, N+32] f16 (transfers
    already queued via copy_to_host_async) -> scalar loss f32."""
    arr_p = np.asarray(out_p).reshape(B, 6, N)
    n_p = _normals_from_covc(arr_p)  # overlaps cloud-g transfer
    arr_g = np.asarray(out_g).reshape(B, 6, N + 32)
    n_g = _normals_from_covc(arr_g[:, :, 0:N])
    dots = (n_p * n_g).sum(-1)
    normc = 1.0 - dots.mean(dtype=np.float64)

    scal = arr_g[:, 0, N:N + 3].astype(np.float64)
    cd = -(scal[:, 0].sum() + scal[:, 2].sum()) / (B * N)
    rep = scal[:, 1].sum() / (B * N * K_REP)

    return np.float32(CD_W * cd + REP_W * rep + NORM_W * normc)


# ============================================================================
# Entry point
# ============================================================================

def kernel(pred, gt):
    pred = np.ascontiguousarray(np.asarray(pred, dtype=np.float32))
    gt = np.ascontiguousarray(np.asarray(gt, dtype=np.float32))
    assert pred.shape == (B, N, DIM) and gt.shape == (B, N, DIM)
    out_p, out_g = _get_runner().run(pred, gt)
    return _host_combine(out_p, out_g)


if __name__ == "__main__":
    rng = np.random.default_rng(0)
    pred = rng.uniform(size=(B, N, DIM)).astype(np.float32)
    gt = rng.uniform(size=(B, N, DIM)).astype(np.float32)
    print("loss:", kernel(pred, gt))


# revision 27
# speedup vs baseline: 1.2490x; 1.2490x over previous
"""Trainium2 Bass kernel for nn_CombinedLoss (chamfer + repulsion + PCA-normal
consistency) on point clouds [8, 2048, 3].

Sharding: data-parallel over batch B=8 across 8 NeuronCores (1 sample/core).

v2 — restructured for the axon tunnel's ~60-80ms/RPC latency (the baseline
spent ~0.95s/call on host-prepped input upload, 6 per-tensor output fetches,
and per-call jit retracing):
  - raw pred/gt uploaded (392KB total); ALL input prep happens on device
    (squared norms, fp32 distance-matmul operand rows, feature rows and
    their bf16 hi/lo transposed layout for the covariance matmul)
  - distance matrices -D via fp32 PE matmuls with K=5 augmented contraction
  - chamfer row/col reductions and the repulsion moment inversion are
    reduced to per-core SCALARS on device
  - per-point 3x3 PCA covariances are centered ON DEVICE and emitted as f16
    [xx,xy,xz,yy,yz,zz] rows in two output tensors (cloud p / cloud g +
    scalars), fetched concurrently (~0.4MB total)
  - the jitted shard_map executable is built once and cached; the donated
    output buffers are recycled from the previous call's outputs
Host: smallest-eigval eigenvectors via a numba scalar port of the fp32
LAPACK-ssyevd sign-convention replica (validated 100% against jax CPU eigh
signs; ~9ms for all 32768 matrices), then the weighted loss.
"""

import numpy as np

try:
    import ml_dtypes

    BF16 = ml_dtypes.bfloat16
except Exception:  # pragma: no cover
    BF16 = None

B, N, DIM = 8, 2048, 3
K_REP = 4
REP_THRESH = np.float32(0.02)
R2 = float(np.float32(REP_THRESH) * np.float32(REP_THRESH))
K_NORM = 16
CD_W, REP_W, NORM_W = 1.0, 0.1, 0.01
NB = N // 128  # 16 row blocks
NEG_BIG = np.float32(-1e30)


# ============================================================================
# Bass device kernel builder
# ============================================================================

def _build_nc(split_waits=True):
    import concourse.bass as bass
    import concourse.mybir as mybir
    from concourse.tile import TileContext

    f32 = mybir.dt.float32
    bf16 = mybir.dt.bfloat16
    Alu = mybir.AluOpType
    Act = mybir.ActivationFunctionType
    Axis = mybir.AxisListType

    nc = bass.Bass()

    # ---- DRAM io (declaration order == jit operand order) ----
    f16 = mybir.dt.float16
    pred = nc.dram_tensor("pred", [N, DIM], f32, kind="ExternalInput")
    gt = nc.dram_tensor("gt", [N, DIM], f32, kind="ExternalInput")
    ident = nc.dram_tensor("ident", [128, 128], bf16, kind="ExternalInput")
    negdiag = nc.dram_tensor("negdiag", [128, 128], bf16, kind="ExternalInput")
    # centered covariance entries [xx,xy,xz,yy,yz,zz] per point, f16;
    # out_g additionally carries the 4 scalars in its last 32 columns
    out_p = nc.dram_tensor("out_p", [6, N], f16, kind="ExternalOutput")
    out_g = nc.dram_tensor("out_g", [6, N + 32], f16, kind="ExternalOutput")

    with TileContext(nc) as tc:
        import contextlib
        ctx = contextlib.ExitStack()
        with ctx:
            persist = ctx.enter_context(tc.tile_pool(name="persist", bufs=1))
            big = ctx.enter_context(tc.tile_pool(name="big", bufs=1))
            scrp = ctx.enter_context(tc.tile_pool(name="scr", bufs=1))
            ndmp = ctx.enter_context(tc.tile_pool(name="ndm", bufs=2))
            wtp = ctx.enter_context(tc.tile_pool(name="wtp", bufs=2))
            psd = ctx.enter_context(tc.tile_pool(name="psd", bufs=2, space="PSUM"))
            psc = ctx.enter_context(tc.tile_pool(name="psc", bufs=1, space="PSUM"))

            # ---- consts ----
            t_ident = persist.tile([128, 128], bf16, tag="ident")
            t_negdiag = persist.tile([128, 128], bf16, tag="ndg")
            nc.sync.dma_start(t_ident[:], ident[:])
            nc.sync.dma_start(t_negdiag[:], negdiag[:])
            t_ones = persist.tile([128, 128], bf16, tag="ones")
            nc.vector.memset(t_ones[:], 1.0)
            t_bias4 = persist.tile([128, 1], f32, tag="bias4")
            t_bias0 = persist.tile([128, 1], f32, tag="bias0")
            nc.vector.memset(t_bias4[:], R2)
            nc.vector.memset(t_bias0[:], 0.0)
            t_ones6 = persist.tile([1, 8], f32, tag="ones6")
            nc.vector.memset(t_ones6[:], 1.0)

            # ---- persistent per-cloud operand tiles ----
            A5 = {}; W5 = {}; F10 = {}; FT = {}
            for cl in ("p", "g"):
                A5[cl] = persist.tile([5, N], f32, tag=f"A5{cl}", name=f"A5{cl}")
                W5[cl] = persist.tile([5, N], f32, tag=f"W5{cl}", name=f"W5{cl}")
                F10[cl] = persist.tile([10, N], f32, tag=f"F10{cl}", name=f"F10{cl}")
                FT[cl] = persist.tile([128, NB * 20], bf16, tag=f"FT{cl}",
                                      name=f"FT{cl}")

            t_rowmax = persist.tile([128, NB, 2], f32, tag="rowmax")
            t_s1 = persist.tile([128, NB], f32, tag="s1")
            t_s2 = persist.tile([128, NB], f32, tag="s2")

            # ================= on-device prep =================
            # A5 = [2x, 2y, 2z, nn, 1] (fp32 matmul lhs rows)
            # W5 = [x, y, z, -1, -nn]  (fp32 matmul rhs rows)
            # F10 = [x2,xy,xz,y2,yz,z2,x,y,z,1] of centered coords
            # FT  = transposed bf16 hi/lo features [128, kb*20 + (0:10 hi|10:20 lo)]
            def prep(src_dram, cl):
                P3 = scrp.tile([3, N], f32, tag="P3")
                nc.sync.dma_start(P3[:], src_dram[:].rearrange("a b -> b a"))
                S3 = scrp.tile([3, N], f32, tag="S3")
                nc.vector.tensor_tensor(S3[:], P3[:], P3[:], Alu.mult)
                r1 = scrp.tile([1, N], f32, tag="r1")
                r2t = scrp.tile([1, N], f32, tag="r2t")
                nc.sync.dma_start(r1[:], S3[1:2, :])
                nc.sync.dma_start(r2t[:], S3[2:3, :])
                nn = scrp.tile([1, N], f32, tag="nn")
                nc.vector.tensor_tensor(nn[:], S3[0:1, :], r1[:], Alu.add)
                nc.vector.tensor_tensor(nn[:], nn[:], r2t[:], Alu.add)
                # engine ops may only start at partitions {0,32,64,96}: memset
                # the whole tile for the constant rows, DMA the odd-row writes
                a5, w5 = A5[cl], W5[cl]
                nc.vector.memset(a5[:], 1.0)
                nc.scalar.activation(a5[0:3, :], P3[:], Act.Copy, scale=2.0)
                nc.sync.dma_start(a5[3:4, :], nn[:])
                nc.vector.memset(w5[:], -1.0)
                nc.vector.tensor_copy(w5[0:3, :], P3[:])
                nnn = scrp.tile([1, N], f32, tag="nnn")
                nc.scalar.activation(nnn[:], nn[:], Act.Copy, scale=-1.0)
                nc.sync.dma_start(w5[4:5, :], nnn[:])
                # centered features
                C3 = scrp.tile([3, N], f32, tag="C3")
                nc.vector.tensor_scalar_add(C3[:], P3[:], -0.5)
                A6 = scrp.tile([6, N], f32, tag="A6")
                B6 = scrp.tile([6, N], f32, tag="B6")
                # A6 rows = [c0,c0,c0,c1,c1,c2]; B6 rows = [c0,c1,c2,c1,c2,c2]
                nc.vector.tensor_copy(A6[0:1, :], C3[0:1, :])
                nc.sync.dma_start(A6[1:2, :], C3[0:1, :])
                nc.sync.dma_start(A6[2:3, :], C3[0:1, :])
                nc.sync.dma_start(A6[3:4, :], C3[1:2, :])
                nc.sync.dma_start(A6[4:5, :], C3[1:2, :])
                nc.sync.dma_start(A6[5:6, :], C3[2:3, :])
                nc.vector.tensor_copy(B6[0:3, :], C3[:])
                nc.sync.dma_start(B6[3:5, :], C3[1:3, :])
                nc.sync.dma_start(B6[5:6, :], C3[2:3, :])
                f10 = F10[cl]
                nc.vector.memset(f10[:], 1.0)
                nc.vector.tensor_tensor(f10[0:6, :], A6[:], B6[:], Alu.mult)
                nc.sync.dma_start(f10[6:9, :], C3[:])
                # bf16 hi/lo split of features
                hi10 = scrp.tile([10, N], bf16, tag="hi10")
                hif = scrp.tile([10, N], f32, tag="hif")
                lo10f = scrp.tile([10, N], f32, tag="lo10f")
                lo10 = scrp.tile([10, N], bf16, tag="lo10")
                nc.scalar.activation(hi10[:], f10[:], Act.Copy)
                nc.scalar.activation(hif[:], hi10[:], Act.Copy)
                nc.vector.tensor_tensor(lo10f[:], f10[:], hif[:], Alu.subtract)
                nc.scalar.activation(lo10[:], lo10f[:], Act.Copy)
                # transpose [10, 128]-chunks -> FT[:, kb*20 + 0:10 / 10:20]
                ftt = FT[cl]
                for b in range(NB):
                    csl = slice(b * 128, (b + 1) * 128)
                    pst = psd.tile([128, 16], bf16, tag="dps")
                    nc.tensor.transpose(pst[:, 0:10], hi10[:, csl],
                                        t_ident[0:10, 0:10])
                    nc.scalar.activation(ftt[:, b * 20:b * 20 + 10], pst[:, 0:10],
                                         Act.Copy)
                    pst2 = psd.tile([128, 16], bf16, tag="dps")
                    nc.tensor.transpose(pst2[:, 0:10], lo10[:, csl],
                                        t_ident[0:10, 0:10])
                    nc.scalar.activation(ftt[:, b * 20 + 10:b * 20 + 20],
                                         pst2[:, 0:10], Act.Copy)

            prep(pred, "p")
            prep(gt, "g")

            # fp32 distance matmul: psum[128, 1024] = -D block (row block b,
            # column half h) between clouds (a5 lhs, w5 rhs)
            def build_half(a5, w5, b, h, ps):
                for j in range(2):
                    nc.tensor.matmul(
                        ps[:, j * 512:(j + 1) * 512],
                        a5[:, b * 128:(b + 1) * 128],
                        w5[:, h * 1024 + j * 512:h * 1024 + (j + 1) * 512],
                        start=True, stop=True,
                    )

            # ================= phase 1: chamfer on -Dpg =================
            t_colacc = big.tile([128, N], f32, tag="bigA")
            t_colred = big.tile([128, N], f32, tag="bigB")
            for b in range(NB):
                for h in range(2):
                    ps = psd.tile([128, 1024], f32, tag="dps")
                    build_half(A5["p"], W5["g"], b, h, ps)
                    nc.vector.tensor_reduce(t_rowmax[:, b, h:h + 1],
                                            ps[:], Axis.X, Alu.max)
                    cslice = slice(h * 1024, (h + 1) * 1024)
                    if b == 0:
                        nc.vector.tensor_copy(t_colacc[:, cslice], ps[:])
                    else:
                        nc.vector.tensor_tensor(t_colacc[:, cslice],
                                                t_colacc[:, cslice], ps[:], Alu.max)
            # partition-tree max 128 -> 1 (DMA crosses partitions, DVE cannot)
            for h in [64, 32, 16, 8, 4, 2, 1]:
                nc.sync.dma_start(t_colred[0:h, :], t_colacc[h:2 * h, :])
                nc.vector.tensor_tensor(t_colacc[0:h, :], t_colacc[0:h, :],
                                        t_colred[0:h, :], Alu.max)
            # chamfer scalars: sum of per-row maxes + sum of col maxes (of -D)
            t_cdcol = persist.tile([1, 1], f32, tag="cdcol")
            nc.vector.tensor_reduce(t_cdcol[:], t_colacc[0:1, :], Axis.X, Alu.add)
            t_rowfull = scrp.tile([128, NB], f32, tag="rowfull")
            nc.vector.tensor_reduce(t_rowfull[:], t_rowmax[:], Axis.X, Alu.max)
            t_cdrow = persist.tile([128, 1], f32, tag="cdrow")
            nc.vector.tensor_reduce(t_cdrow[:], t_rowfull[:], Axis.X, Alu.add)

            # ================= phases 2-4 for pp and gg =================
            def normals_phase(cl, out_dram, do_rep):
                a5, w5, ftt, f10 = A5[cl], W5[cl], FT[cl], F10[cl]

                def build_ndm(b):
                    ndm = ndmp.tile([128, N], bf16, tag="ndm", name=f"ndm{cl}{b}")
                    for h in range(2):
                        ps = psd.tile([128, 1024], f32, tag="dps")
                        build_half(a5, w5, b, h, ps)
                        nc.scalar.activation(ndm[:, h * 1024:(h + 1) * 1024],
                                             ps[:], Act.Copy)
                    nc.vector.tensor_tensor(
                        ndm[:, b * 128:(b + 1) * 128],
                        ndm[:, b * 128:(b + 1) * 128],
                        t_negdiag[:], Alu.add)
                    return ndm

                # pass 1: repulsion moments + 16-NN radius (tau) per row
                t_tau = scrp.tile([128, NB], f32, tag="tau")
                for b in range(NB):
                    ndm = build_ndm(b)
                    if do_rep:
                        scr = scrp.tile([128, N], bf16, tag="repscr")
                        scr2 = scrp.tile([128, N], bf16, tag="repscr2")
                        nc.scalar.activation(scr[:], ndm[:], Act.Relu,
                                             bias=t_bias4[:],
                                             accum_out=t_s1[:, b:b + 1])
                        nc.scalar.activation(scr2[:], scr[:], Act.Square,
                                             bias=t_bias0[:],
                                             accum_out=t_s2[:, b:b + 1])
                    t1 = scrp.tile([128, 1024], bf16, tag="tree1")
                    At = scrp.tile([128, 512], bf16, tag="treeA")
                    At2 = scrp.tile([128, 512], bf16, tag="treeA2")
                    m8a = scrp.tile([128, 8], bf16, tag="m8a")
                    m8b = scrp.tile([128, 8], bf16, tag="m8b")
                    nc.vector.tensor_tensor(t1[:], ndm[:, 0:1024],
                                            ndm[:, 1024:2048], Alu.max)
                    nc.vector.tensor_tensor(At[:], t1[:, 0:512],
                                            t1[:, 512:1024], Alu.max)
                    nc.vector.max(m8a[:], At[:])
                    nc.vector.match_replace(At2[:], m8a[:], At[:], float(NEG_BIG))
                    nc.vector.max(m8b[:], At2[:])
                    nc.vector.tensor_copy(t_tau[:, b:b + 1], m8b[:, 6:7])

                # tau broadcast: per-row tau -> [1, N] row -> PE ones-matmul
                # broadcast across partitions; mask compare is then direct on
                # the SYMMETRIC ndm blocks: wt[j, i] = (ndm[j, i] >= tau_i)
                t_taub = scrp.tile([128, 128], bf16, tag="taub")
                nc.vector.memset(t_taub[:], 0.0)
                nc.vector.tensor_copy(t_taub[:, 0:NB], t_tau[:])
                ps_tt = psd.tile([128, 128], bf16, tag="dps")
                nc.tensor.transpose(ps_tt[:], t_taub[:], t_ident[:])
                t_tt = scrp.tile([NB, 128], bf16, tag="tts")
                nc.scalar.activation(t_tt[:], ps_tt[0:NB, :], Act.Copy)
                t_tauT = scrp.tile([128, N], bf16, tag="tauT")
                nc.vector.memset(t_tauT[:], 0.0)
                nc.sync.dma_start(t_tauT[0:1, :], t_tt[:])
                t_taubc = scrp.tile([128, N], bf16, tag="taubc")
                for h in range(2):
                    ps_tau = psd.tile([128, 1024], f32, tag="dps")
                    for bb in range(8):
                        c0 = h * 1024 + bb * 128
                        nc.tensor.matmul(ps_tau[:, bb * 128:(bb + 1) * 128],
                                         t_ones[:],
                                         t_tauT[:, c0:c0 + 128],
                                         start=True, stop=True)
                    nc.scalar.activation(t_taubc[:, h * 1024:(h + 1) * 1024],
                                         ps_tau[:], Act.Copy)

                # pass 2: rebuild -D per block, mask, accumulate covariance
                # moments cps[10, N] over kb (hi+lo)
                cps = psc.tile([10, N], f32, tag="cps")
                for kb in range(NB):
                    ndm = build_ndm(kb)
                    wt = wtp.tile([128, N], bf16, tag="wt", name=f"wt{cl}{kb}")
                    nc.vector.tensor_tensor(wt[:], ndm[:], t_taubc[:], Alu.is_ge)
                    for j in range(4):
                        cols = slice(j * 512, (j + 1) * 512)
                        for half in range(2):
                            nc.tensor.matmul(
                                cps[:, cols],
                                ftt[:, kb * 20 + half * 10:kb * 20 + (half + 1) * 10],
                                wt[:, cols],
                                start=(kb == 0 and half == 0),
                                stop=(kb == NB - 1 and half == 1))
                # self add, then center on device:
                #   covc[ab] = M2[ab]/cnt - (s[a]/cnt)*(s[b]/cnt)   (f16 out)
                covsb = big.tile([10, N], f32, tag="bigA", name=f"covsb{cl}")
                nc.vector.tensor_tensor(covsb[:], cps[:], f10[:], Alu.add)
                rr = scrp.tile([1, N], f32, tag="r1")
                nc.sync.dma_start(rr[:], covsb[9:10, :])
                rcp = scrp.tile([1, N], f32, tag="r2t")
                nc.vector.reciprocal(rcp[:], rr[:])
                mus = scrp.tile([3, N], f32, tag="S3")
                nc.sync.dma_start(mus[:], covsb[6:9, :])
                psB3 = psc.tile([3, N], f32, tag="cps", name=f"psB3{cl}")
                for j in range(4):
                    cj = slice(j * 512, (j + 1) * 512)
                    nc.tensor.matmul(psB3[:, cj], t_ones6[0:1, 0:3], rcp[:, cj],
                                     start=True, stop=True)
                mu3 = scrp.tile([3, N], f32, tag="C3")
                nc.vector.tensor_tensor(mu3[:], mus[:], psB3[:], Alu.mult)
                A6m = scrp.tile([6, N], f32, tag="A6")
                B6m = scrp.tile([6, N], f32, tag="B6")
                nc.vector.tensor_copy(A6m[0:1, :], mu3[0:1, :])
                nc.sync.dma_start(A6m[1:2, :], mu3[0:1, :])
                nc.sync.dma_start(A6m[2:3, :], mu3[0:1, :])
                nc.sync.dma_start(A6m[3:4, :], mu3[1:2, :])
                nc.sync.dma_start(A6m[4:5, :], mu3[1:2, :])
                nc.sync.dma_start(A6m[5:6, :], mu3[2:3, :])
                nc.vector.tensor_copy(B6m[0:3, :], mu3[:])
                nc.sync.dma_start(B6m[3:5, :], mu3[1:3, :])
                nc.sync.dma_start(B6m[5:6, :], mu3[2:3, :])
                P6 = scrp.tile([6, N], f32, tag="lo10f")
                nc.vector.tensor_tensor(P6[:], A6m[:], B6m[:], Alu.mult)
                psB6 = psc.tile([6, N], f32, tag="cps", name=f"psB6{cl}")
                for j in range(4):
                    cj = slice(j * 512, (j + 1) * 512)
                    nc.tensor.matmul(psB6[:, cj], t_ones6[0:1, 0:6], rcp[:, cj],
                                     start=True, stop=True)
                M2r = scrp.tile([6, N], f32, tag="hif")
                nc.vector.tensor_tensor(M2r[:], covsb[0:6, :], psB6[:], Alu.mult)
                covc = scrp.tile([6, N], f16, tag="hi10")
                nc.vector.tensor_tensor(covc[:], M2r[:], P6[:], Alu.subtract)
                nc.sync.dma_start(out_dram[:, 0:N], covc[:])

            normals_phase("p", out_p, do_rep=True)

            # ---- repulsion moment inversion -> per-row contribution ----
            # a,b = (s1 +- sqrt(2*s2 - s1^2))/2; d=sqrt(r2-v); contrib =
            # relu(0.02-da)+relu(0.02-db), gated by s1>0
            sh = [128, NB]
            t_t1 = scrp.tile(sh, f32, tag="rp1")
            t_t2 = scrp.tile(sh, f32, tag="rp2")
            t_sq = scrp.tile(sh, f32, tag="rp3")
            t_va = scrp.tile(sh, f32, tag="rp4")
            t_vb = scrp.tile(sh, f32, tag="rp5")
            t_ca = scrp.tile(sh, f32, tag="rp6")
            t_cb = scrp.tile(sh, f32, tag="rp7")
            t_msk = scrp.tile(sh, f32, tag="rp8")
            Alu_ = Alu
            nc.vector.tensor_tensor(t_t1[:], t_s1[:], t_s1[:], Alu_.mult)
            nc.vector.tensor_scalar(t_t2[:], t_s2[:], 2.0, None, Alu_.mult)
            nc.vector.tensor_tensor(t_t2[:], t_t2[:], t_t1[:], Alu_.subtract)
            nc.vector.tensor_scalar_max(t_t2[:], t_t2[:], 0.0)
            nc.scalar.activation(t_sq[:], t_t2[:], Act.Sqrt)
            nc.vector.tensor_tensor(t_va[:], t_s1[:], t_sq[:], Alu_.add)
            nc.vector.tensor_scalar(t_va[:], t_va[:], 0.5, R2, Alu_.mult, Alu_.min)
            nc.vector.tensor_tensor(t_vb[:], t_s1[:], t_sq[:], Alu_.subtract)
            nc.vector.tensor_scalar(t_vb[:], t_vb[:], 0.5, 0.0, Alu_.mult, Alu_.max)
            # da = sqrt(max(r2 - va, 1e-12)); contrib_a = max(0.02 - da, 0)
            for tv, tc_ in ((t_va, t_ca), (t_vb, t_cb)):
                nc.vector.tensor_scalar(tv[:], tv[:], -1.0, R2, Alu_.mult, Alu_.add)
                nc.vector.tensor_scalar_max(tv[:], tv[:], 1e-12)
                nc.scalar.activation(tv[:], tv[:], Act.Sqrt)
                nc.vector.tensor_scalar(tc_[:], tv[:], -1.0, float(REP_THRESH),
                                        Alu_.mult, Alu_.add)
                nc.vector.tensor_scalar_max(tc_[:], tc_[:], 0.0)
            nc.vector.tensor_scalar(t_msk[:], t_s1[:], 0.0, None, Alu_.is_gt)
            nc.vector.tensor_tensor(t_ca[:], t_ca[:], t_cb[:], Alu_.add)
            nc.vector.tensor_tensor(t_ca[:], t_ca[:], t_msk[:], Alu_.mult)
            t_reprow = persist.tile([128, 1], f32, tag="reprow")
            nc.vector.tensor_reduce(t_reprow[:], t_ca[:], Axis.X, Alu_.add)

            # ---- partition-sum [cd_row, rep] via DMA tree; pack scalars ----
            t_P2 = scrp.tile([128, 2], f32, tag="P2")
            t_P2s = scrp.tile([64, 2], f32, tag="P2s")
            nc.vector.tensor_copy(t_P2[:, 0:1], t_cdrow[:])
            nc.vector.tensor_copy(t_P2[:, 1:2], t_reprow[:])
            for h in [64, 32, 16, 8, 4, 2, 1]:
                nc.sync.dma_start(t_P2s[0:h, :], t_P2[h:2 * h, :])
                nc.vector.tensor_tensor(t_P2[0:h, :], t_P2[0:h, :],
                                        t_P2s[0:h, :], Alu.add)
            t_z6 = scrp.tile([6, 32], f16, tag="z6")
            nc.vector.memset(t_z6[:], 0.0)
            nc.vector.tensor_copy(t_z6[0:1, 0:2], t_P2[0:1, :])
            nc.vector.tensor_copy(t_z6[0:1, 2:3], t_cdcol[:])
            nc.sync.dma_start(out_g[:, N:N + 32], t_z6[:])

            normals_phase("g", out_g, do_rep=False)

    if split_waits:
        _split_excess_waits(nc, mybir)
    return nc


def _split_excess_waits(nc, mybir, max_w=1, max_u=1):
    """This toolchain's walrus accepts at most 1 sync wait and 1 update per
    instruction. Move excess waits onto same-engine prefix NoOps (the engine
    is in-order, so waiting earlier is equivalent) and excess updates onto
    suffix NoOps (signalling marginally later is safe)."""
    n = 0
    for func in nc.m.functions:
        for block in func.blocks:
            lst = block.instructions
            new = []
            for inst in lst:
                si = inst.sync_info
                ow = list(si.on_wait) if (si and si.on_wait) else []
                if len(ow) > max_w:
                    extra, keep = ow[:-max_w], ow[-max_w:]
                    for k in range(0, len(extra), max_w):
                        nop = mybir.InstNoOp(name=f"I-wsplit-{n}"); n += 1
                        nop.engine = inst.engine
                        nop.sync_info = mybir.SyncInfo(
                            on_wait=extra[k:k + max_w], on_update=[])
                        new.append(nop)
                    si.on_wait = keep
                new.append(inst)
                ou = list(si.on_update) if (si and si.on_update) else []
                if len(ou) > max_u:
                    keep_u, extra_u = ou[:max_u], ou[max_u:]
                    si.on_update = keep_u
                    for k in range(0, len(extra_u), max_u):
                        nop = mybir.InstNoOp(name=f"I-usplit-{n}"); n += 1
                        nop.engine = inst.engine
                        nop.sync_info = mybir.SyncInfo(
                            on_wait=[], on_update=extra_u[k:k + max_u])
                        new.append(nop)
            lst[:] = new
    return n


_NC_CACHE = None


def _get_nc():
    global _NC_CACHE
    if _NC_CACHE is None:
        _NC_CACHE = _build_nc()
    return _NC_CACHE


def _consts_np():
    negdiag = np.zeros((128, 128), dtype=BF16)
    np.fill_diagonal(negdiag, BF16(NEG_BIG))
    ident = np.zeros((128, 128), dtype=BF16)
    np.fill_diagonal(ident, BF16(1.0))
    return ident, negdiag


# ============================================================================
# Cached jit runner (replicates bass2jax.run_bass_via_pjrt, but the jitted
# executable, mesh, and const device buffers are built ONCE; the donated
# output buffer is recycled from the previous call's output)
# ============================================================================

class _Runner:
    def __init__(self):
        import jax
        from jax.sharding import Mesh, PartitionSpec, NamedSharding
        from jax.experimental.shard_map import shard_map
        from concourse import bass2jax
        import concourse.mybir as mybir

        self.jax = jax
        nc = _get_nc()
        bass2jax.install_neuronx_cc_hook()

        partition_name = (nc.partition_id_tensor.name
                          if nc.partition_id_tensor else None)
        in_names, out_names, out_avals, zero_outs = [], [], [], []
        for alloc in nc.m.functions[0].allocations:
            if not isinstance(alloc, mybir.MemoryLocationSet):
                continue
            name = alloc.memorylocations[0].name
            if alloc.kind == "ExternalInput":
                if name != partition_name:
                    in_names.append(name)
            elif alloc.kind == "ExternalOutput":
                shape = tuple(alloc.tensor_shape)
                dtype = mybir.dt.np(alloc.dtype)
                out_names.append(name)
                out_avals.append(jax.core.ShapedArray(shape, dtype))
                zero_outs.append((shape, dtype))
        assert in_names == ["pred", "gt", "ident", "negdiag"], in_names
        assert out_names == ["out_p", "out_g"], out_names
        n_params = len(in_names)
        n_outs = len(out_names)
        all_names = in_names + out_names
        if partition_name is not None:
            all_names.append(partition_name)
        self.zero_outs = zero_outs

        def _body(*args):
            operands = list(args)
            if partition_name is not None:
                operands.append(bass2jax.partition_id_tensor())
            outs = bass2jax._bass_exec_p.bind(
                *operands,
                out_avals=tuple(out_avals),
                in_names=tuple(all_names),
                out_names=tuple(out_names),
                lowering_input_output_aliases=(),
                sim_require_finite=True,
                sim_require_nnan=True,
                nc=nc,
            )
            return tuple(outs)

        devices = jax.devices()[:B]
        assert len(devices) == B, f"need {B} devices, have {len(jax.devices())}"
        mesh = Mesh(np.asarray(devices), ("core",))
        pspec = PartitionSpec("core")
        self._fn = jax.jit(
            shard_map(_body, mesh=mesh,
                      in_specs=(pspec,) * (n_params + n_outs),
                      out_specs=(pspec,) * n_outs,
                      check_rep=False),
            donate_argnums=tuple(range(n_params, n_params + n_outs)),
            keep_unused=True,
        )
        ident, negdiag = _consts_np()
        sh = NamedSharding(mesh, pspec)
        self._ident = jax.device_put(np.tile(ident, (B, 1)), sh)
        self._negdiag = jax.device_put(np.tile(negdiag, (B, 1)), sh)
        self._donate = None  # recycled output buffers

    def run(self, pred, gt):
        """pred, gt: [B, N, 3] f32 -> (fut_p, fut_g) resolving to host
        np.ndarrays [B*6, N] / [B*6, N+32] f16."""
        zeros = self._donate
        if zeros is None:
            zeros = [np.zeros((B * s[0],) + s[1:], d)
                     for s, d in self.zero_outs]
        out_p, out_g = self._fn(pred.reshape(B * N, DIM),
                                gt.reshape(B * N, DIM),
                                self._ident, self._negdiag, *zeros)
        # queue both host transfers EAGERLY: an np.asarray issued after the
        # ready notification pays a full extra tunnel round trip (~100ms);
        # copy_to_host_async rides the execute pipeline instead, and cloud
        # g's transfer proceeds in background while the host runs cloud p's
        # eigensolve
        try:
            out_p.copy_to_host_async()
            out_g.copy_to_host_async()
        except Exception:
            pass
        # the kernel writes every element of both outputs, so last call's
        # outputs can be donated as the next call's output buffers
        self._donate = [out_p, out_g]
        return out_p, out_g


_RUNNER = None


def _get_runner():
    global _RUNNER
    if _RUNNER is None:
        _RUNNER = _Runner()
    return _RUNNER


# ============================================================================
# Host combine
# ============================================================================

# ----------------------------------------------------------------------------
# LAPACK ssyevd 3x3 sign-convention replication (fp32), numba scalar port of
# the vectorized replica validated 100% against jax/scipy CPU eigh signs.
# Falls back to np.linalg.eigh (99.35% sign agreement) without numba.
# ----------------------------------------------------------------------------
try:
    from numba import njit as _njit
    _HAVE_NUMBA = True
except Exception:  # pragma: no cover
    _HAVE_NUMBA = False

if _HAVE_NUMBA:
    _F = np.float32
    _EPS = _F(2.0) ** _F(-24)
    _EPS2 = _F(_EPS * _EPS)
    _SAFMIN = _F(1.1754943508222875e-38)
    _ONE = _F(1.0)
    _TWO = _F(2.0)
    _HALF = _F(0.5)
    _ZERO = _F(0.0)

    @_njit(cache=True, fastmath=False)
    def _fsign(a, b):
        return np.abs(a) if b >= _ZERO else -np.abs(a)

    @_njit(cache=True, fastmath=False)
    def _slapy2(x, y):
        ax = np.abs(x); ay = np.abs(y)
        w = max(ax, ay); z = min(ax, ay)
        if z == _ZERO:
            return w
        r = z / w
        return w * np.sqrt(_ONE + r * r)

    @_njit(cache=True, fastmath=False)
    def _slartg(f, g):
        if g == _ZERO:
            return _ONE, _ZERO, f
        if f == _ZERO:
            return _ZERO, _fsign(_ONE, g), np.abs(g)
        d = np.sqrt(f * f + g * g)
        cs = np.abs(f) / d
        r = _fsign(d, f)
        sn = g / r
        return cs, sn, r

    @_njit(cache=True, fastmath=False)
    def _slaev2(a, b, c):
        sm = a + c
        df = a - c
        adf = np.abs(df)
        tb = b + b
        ab_ = np.abs(tb)
        if np.abs(a) > np.abs(c):
            acmx = a; acmn = c
        else:
            acmx = c; acmn = a
        if adf > ab_:
            r_ = ab_ / adf
            rt = adf * np.sqrt(_ONE + r_ * r_)
        elif adf < ab_:
            r_ = adf / ab_
            rt = ab_ * np.sqrt(_ONE + r_ * r_)
        else:
            rt = ab_ * np.sqrt(_TWO)
        if sm < _ZERO:
            rt1 = _HALF * (sm - rt)
            sgn1 = -_ONE
            rt2 = (acmx / rt1) * acmn - (b / rt1) * b
        elif sm > _ZERO:
            rt1 = _HALF * (sm + rt)
            sgn1 = _ONE
            rt2 = (acmx / rt1) * acmn - (b / rt1) * b
        else:
            rt1 = _HALF * rt
            sgn1 = _ONE
            rt2 = -_HALF * rt
        if df >= _ZERO:
            cs = df + rt
            sgn2 = _ONE
        else:
            cs = df - rt
            sgn2 = -_ONE
        acs = np.abs(cs)
        if acs > ab_:
            ct = -tb / cs
            sn1 = _ONE / np.sqrt(_ONE + ct * ct)
            cs1 = ct * sn1
        else:
            if ab_ == _ZERO:
                cs1 = _ONE
                sn1 = _ZERO
            else:
                tn = -cs / tb
                cs1 = _ONE / np.sqrt(_ONE + tn * tn)
                sn1 = tn * cs1
        if sgn1 == sgn2:
            t = cs1
            cs1 = -sn1
            sn1 = t
        return rt1, rt2, cs1, sn1

    @_njit(cache=True, fastmath=False)
    def _rot(Z, ca, cb, c, s):
        for i in range(3):
            temp = Z[i, cb]
            Z[i, cb] = c * temp - s * Z[i, ca]
            Z[i, ca] = s * temp + c * Z[i, ca]

    @_njit(cache=True, fastmath=False)
    def _eigh3_batch(cv, out):
        # cv: [Bc, 6, N] f32 rows (xx, xy, xz, yy, yz, zz); out: [Bc*N, 3]
        Z = np.empty((3, 3), np.float32)
        n_pts = cv.shape[2]
        for idx in range(cv.shape[0] * n_pts):
            bb = idx // n_pts
            nn_ = idx - bb * n_pts
            a00 = cv[bb, 0, nn_]; a10 = cv[bb, 1, nn_]; a20 = cv[bb, 2, nn_]
            a11 = cv[bb, 3, nn_]; a21 = cv[bb, 4, nn_]; a22 = cv[bb, 5, nn_]
            # ssytd2 lower
            xnorm = np.abs(a20)
            alpha = a10
            beta = -_fsign(_slapy2(alpha, xnorm), alpha)
            refl = xnorm != _ZERO
            if refl:
                tau1 = (beta - alpha) / beta
                v2 = a20 / (alpha - beta)
                w1 = tau1 * a11 + tau1 * (a21 * v2)
                w2 = tau1 * a21 + (tau1 * v2) * a22
                alp = -_HALF * tau1 * (w1 + w2 * v2)
                w1 = w1 + alp
                w2 = w2 + alp * v2
                d0 = a00
                d1 = a11 - (w1 + w1)
                d2 = a22 - ((v2 * w2) + (v2 * w2))
                e0 = beta
                e1 = a21 - (v2 * w1 + w2)
            else:
                tau1 = _ZERO
                v2 = _ZERO
                d0 = a00; d1 = a11; d2 = a22
                e0 = a10; e1 = a21
            for i in range(3):
                for j in range(3):
                    Z[i, j] = _ONE if i == j else _ZERO
            s0 = np.abs(e0) <= (np.sqrt(np.abs(d0)) * np.sqrt(np.abs(d1))) * _EPS
            s1m = np.abs(e1) <= (np.sqrt(np.abs(d1)) * np.sqrt(np.abs(d2))) * _EPS
            if s0:
                e0 = _ZERO
            if s1m:
                e1 = _ZERO
            if s0 and not s1m:
                tst = e1 * e1
                thr = (_EPS2 * np.abs(d1)) * np.abs(d2) + _SAFMIN
                if tst > thr:
                    rt1, rt2, c, s = _slaev2(d1, e1, d2)
                    _rot(Z, 1, 2, c, s)
                    d1 = rt1; d2 = rt2
                e1 = _ZERO
            elif (not s0) and s1m:
                tst = e0 * e0
                thr = (_EPS2 * np.abs(d0)) * np.abs(d1) + _SAFMIN
                if tst > thr:
                    rt1, rt2, c, s = _slaev2(d0, e0, d1)
                    _rot(Z, 0, 1, c, s)
                    d0 = rt1; d1 = rt2
                e0 = _ZERO
            elif (not s0) and (not s1m):
                if np.abs(d2) < np.abs(d0):
                    # QR variant
                    l = 2
                    for _it in range(40):
                        if l <= -1:
                            break
                        if l == 2:
                            m2s = e1 * e1 <= (_EPS2 * np.abs(d2)) * np.abs(d1) + _SAFMIN
                            m1s = e0 * e0 <= (_EPS2 * np.abs(d1)) * np.abs(d0) + _SAFMIN
                            if m2s:
                                e1 = _ZERO
                                l = 1
                            elif m1s:
                                e0 = _ZERO
                                rt1, rt2, c, s = _slaev2(d1, e1, d2)
                                _rot(Z, 1, 2, c, s)
                                d1 = rt1; d2 = rt2
                                e1 = _ZERO
                                l = 0
                            else:
                                P = d2
                                G = (d1 - P) / (_TWO * e1)
                                R = _slapy2(G, _ONE)
                                G = d0 - P + (e1 / (G + _fsign(R, G)))
                                Fv = e0
                                Bv = e0
                                C, S, R = _slartg(G, Fv)
                                G2 = d0
                                R = (d1 - G2) * S + (_TWO * C) * Bv
                                Pv = S * R
                                d0n = G2 + Pv
                                G = C * R - Bv
                                c0 = C; s0_ = S
                                Fv = S * e1
                                Bv = C * e1
                                C, S, R = _slartg(G, Fv)
                                e0n = R
                                G2 = d1 - Pv
                                R = (d2 - G2) * S + (_TWO * C) * Bv
                                Pv2 = S * R
                                d1n = G2 + Pv2
                                G = C * R - Bv
                                c1 = C; s1_ = S
                                _rot(Z, 0, 1, c0, s0_)
                                _rot(Z, 1, 2, c1, s1_)
                                d0 = d0n; d1 = d1n; d2 = d2 - Pv2
                                e0 = e0n; e1 = G
                        elif l == 1:
                            ms = e0 * e0 <= (_EPS2 * np.abs(d1)) * np.abs(d0) + _SAFMIN
                            if ms:
                                e0 = _ZERO
                                l = 0
                            else:
                                rt1, rt2, c, s = _slaev2(d0, e0, d1)
                                _rot(Z, 0, 1, c, s)
                                d0 = rt1; d1 = rt2
                                e0 = _ZERO
                                l = -1
                        else:  # l == 0
                            l = -1
                else:
                    # QL variant
                    l = 0
                    for _it in range(40):
                        if l >= 3:
                            break
                        if l == 0:
                            m0s = e0 * e0 <= (_EPS2 * np.abs(d0)) * np.abs(d1) + _SAFMIN
                            m1s = e1 * e1 <= (_EPS2 * np.abs(d1)) * np.abs(d2) + _SAFMIN
                            if m0s:
                                e0 = _ZERO
                                l = 1
                            elif m1s:
                                e1 = _ZERO
                                rt1, rt2, c, s = _slaev2(d0, e0, d1)
                                _rot(Z, 0, 1, c, s)
                                d0 = rt1; d1 = rt2
                                e0 = _ZERO
                                l = 2
                            else:
                                P = d0
                                G = (d1 - P) / (_TWO * e0)
                                R = _slapy2(G, _ONE)
                                G = d2 - P + (e0 / (G + _fsign(R, G)))
                                Fv = e1
                                Bv = e1
                                C, S, R = _slartg(G, Fv)
                                G2 = d2
                                R = (d1 - G2) * S + (_TWO * C) * Bv
                                Pv = S * R
                                d2n = G2 + Pv
                                G = C * R - Bv
                                c1 = C; s1_ = -S
                                Fv = S * e0
                                Bv = C * e0
                                C, S, R = _slartg(G, Fv)
                                e1n = R
                                G2 = d1 - Pv
                                R = (d0 - G2) * S + (_TWO * C) * Bv
                                Pv2 = S * R
                                d1n = G2 + Pv2
                                G = C * R - Bv
                                c0 = C; s0_ = -S
                                _rot(Z, 1, 2, c1, s1_)
                                _rot(Z, 0, 1, c0, s0_)
                                d2 = d2n; d1 = d1n; d0 = d0 - Pv2
                                e1 = e1n; e0 = G
                        elif l == 1:
                            ms = e1 * e1 <= (_EPS2 * np.abs(d1)) * np.abs(d2) + _SAFMIN
                            if ms:
                                e1 = _ZERO
                                l = 2
                            else:
                                rt1, rt2, c, s = _slaev2(d1, e1, d2)
                                _rot(Z, 1, 2, c, s)
                                d1 = rt1; d2 = rt2
                                e1 = _ZERO
                                l = 3
                        else:  # l == 2
                            l = 3
            # sort eigenvalues ascending, swapping Z columns (ssteqr tail)
            D0 = d0; D1 = d1; D2 = d2
            for i in range(2):
                if i == 0:
                    k = 0; P = D0
                    if D1 < P:
                        k = 1; P = D1
                    if D2 < P:
                        k = 2; P = D2
                    if k != 0:
                        if k == 1:
                            D1 = D0
                        else:
                            D2 = D0
                        D0 = P
                        for r_i in range(3):
                            t = Z[r_i, 0]; Z[r_i, 0] = Z[r_i, k]; Z[r_i, k] = t
                else:
                    if D2 < D1:
                        t2 = D1; D1 = D2; D2 = t2
                        for r_i in range(3):
                            t = Z[r_i, 1]; Z[r_i, 1] = Z[r_i, 2]; Z[r_i, 2] = t
            # back-transform the householder (sorm2r)
            if refl:
                for col in range(3):
                    w = Z[1, col] + v2 * Z[2, col]
                    Z[1, col] = Z[1, col] - tau1 * w
                    Z[2, col] = Z[2, col] - (tau1 * v2) * w
            out[idx, 0] = Z[0, 0]
            out[idx, 1] = Z[1, 0]
            out[idx, 2] = Z[2, 0]


def _normals_from_covc(cv):
    """cv: [B, 6, N] centered covariance rows [xx,xy,xz,yy,yz,zz] (f16) ->
    [B*N, 3] smallest-eigval eigenvectors with ssyevd sign convention."""
    f32 = np.float32
    cv32 = cv.astype(f32)
    if _HAVE_NUMBA:
        out = np.empty((cv32.shape[0] * cv32.shape[2], 3), f32)
        _eigh3_batch(cv32, out)
        return out
    flat = np.ascontiguousarray(cv32.transpose(0, 2, 1).reshape(-1, 6))
    cov = np.empty((flat.shape[0], 3, 3), dtype=f32)
    cov[:, 0, 0] = flat[:, 0]
    cov[:, 0, 1] = cov[:, 1, 0] = flat[:, 1]
    cov[:, 0, 2] = cov[:, 2, 0] = flat[:, 2]
    cov[:, 1, 1] = flat[:, 3]
    cov[:, 1, 2] = cov[:, 2, 1] = flat[:, 4]
    cov[:, 2, 2] = flat[:, 5]
    return np.linalg.eigh(cov)[1][:, :, 0]


def _host_combine(out_p, out_g):
    """out_p/out_g: device outputs [B*6, N] / [B*6, N+32] f16 (transfers
    already queued via copy_to_host_async) -> scalar loss f32."""
    arr_p = np.asarray(out_p).reshape(B, 6, N)
    n_p = _normals_from_covc(arr_p)  # overlaps cloud-g transfer
    arr_g = np.asarray(out_g).reshape(B, 6, N + 32)
    n_g = _normals_from_covc(arr_g[:, :, 0:N])
    dots = (n_p * n_g).sum(-1)
    normc = 1.0 - dots.mean(dtype=np.float64)

    scal = arr_g[:, 0, N:N + 3].astype(np.float64)
    cd = -(scal[:, 0].sum() + scal[:, 2].sum()) / (B * N)
    rep = scal[:, 1].sum() / (B * N * K_REP)

    return np.float32(CD_W * cd + REP_W * rep + NORM_W * normc)


# ============================================================================
# Entry point
# ============================================================================

def kernel(pred, gt):
    pred = np.ascontiguousarray(np.asarray(pred, dtype=np.float32))
    gt = np.ascontiguousarray(np.asarray(gt, dtype=np.float32))
    assert pred.shape == (B, N, DIM) and gt.shape == (B, N, DIM)
    out_p, out_g = _get_runner().run(pred, gt)
    return _host_combine(out_p, out_g)


if __name__ == "__main__":
    rng = np.random.default_rng(0)
    pred = rng.uniform(size=(B, N, DIM)).astype(np.float32)
    gt = rng.uniform(size=(B, N, DIM)).astype(np.float32)
    print("loss:", kernel(pred, gt))


# revision 31
# speedup vs baseline: 1.3571x; 1.0866x over previous
"""Trainium2 Bass kernel for nn_CombinedLoss (chamfer + repulsion + PCA-normal
consistency) on point clouds [8, 2048, 3].

Sharding: data-parallel over batch B=8 across 8 NeuronCores (1 sample/core).

v2 — restructured for the axon tunnel's ~60-80ms/RPC latency (the baseline
spent ~0.95s/call on host-prepped input upload, 6 per-tensor output fetches,
and per-call jit retracing):
  - raw pred/gt uploaded (392KB total); ALL input prep happens on device
    (squared norms, fp32 distance-matmul operand rows, feature rows and
    their bf16 hi/lo transposed layout for the covariance matmul)
  - distance matrices -D via fp32 PE matmuls with K=5 augmented contraction
  - chamfer row/col reductions and the repulsion moment inversion are
    reduced to per-core SCALARS on device
  - per-point 3x3 PCA covariances are centered ON DEVICE and emitted as f16
    [xx,xy,xz,yy,yz,zz] rows in two output tensors (cloud p / cloud g +
    scalars), fetched concurrently (~0.4MB total)
  - the jitted shard_map executable is built once and cached; the donated
    output buffers are recycled from the previous call's outputs
Host: smallest-eigval eigenvectors via a numba scalar port of the fp32
LAPACK-ssyevd sign-convention replica (validated 100% against jax CPU eigh
signs; ~9ms for all 32768 matrices), then the weighted loss.
"""

import numpy as np

try:
    import ml_dtypes

    BF16 = ml_dtypes.bfloat16
except Exception:  # pragma: no cover
    BF16 = None

B, N, DIM = 8, 2048, 3
K_REP = 4
REP_THRESH = np.float32(0.02)
R2 = float(np.float32(REP_THRESH) * np.float32(REP_THRESH))
K_NORM = 16
CD_W, REP_W, NORM_W = 1.0, 0.1, 0.01
NB = N // 128  # 16 row blocks
NEG_BIG = np.float32(-1e30)


# ============================================================================
# Bass device kernel builder
# ============================================================================

def _build_nc(split_waits=True):
    import concourse.bass as bass
    import concourse.mybir as mybir
    from concourse.tile import TileContext

    f32 = mybir.dt.float32
    bf16 = mybir.dt.bfloat16
    Alu = mybir.AluOpType
    Act = mybir.ActivationFunctionType
    Axis = mybir.AxisListType

    nc = bass.Bass()

    # ---- DRAM io (declaration order == jit operand order) ----
    f16 = mybir.dt.float16
    pred = nc.dram_tensor("pred", [N, DIM], f32, kind="ExternalInput")
    gt = nc.dram_tensor("gt", [N, DIM], f32, kind="ExternalInput")
    ident = nc.dram_tensor("ident", [128, 128], bf16, kind="ExternalInput")
    negdiag = nc.dram_tensor("negdiag", [128, 128], bf16, kind="ExternalInput")
    # centered covariance entries [xx,xy,xz,yy,yz,zz] per point, f16;
    # out_g additionally carries the 4 scalars in its last 32 columns
    out_p = nc.dram_tensor("out_p", [6, N], f16, kind="ExternalOutput")
    out_g = nc.dram_tensor("out_g", [6, N + 32], f16, kind="ExternalOutput")

    with TileContext(nc) as tc:
        import contextlib
        ctx = contextlib.ExitStack()
        with ctx:
            persist = ctx.enter_context(tc.tile_pool(name="persist", bufs=1))
            big = ctx.enter_context(tc.tile_pool(name="big", bufs=1))
            scrp = ctx.enter_context(tc.tile_pool(name="scr", bufs=1))
            ndmp = ctx.enter_context(tc.tile_pool(name="ndm", bufs=2))
            wtp = ctx.enter_context(tc.tile_pool(name="wtp", bufs=2))
            psd = ctx.enter_context(tc.tile_pool(name="psd", bufs=2, space="PSUM"))
            psc = ctx.enter_context(tc.tile_pool(name="psc", bufs=1, space="PSUM"))

            # ---- consts ----
            t_ident = persist.tile([128, 128], bf16, tag="ident")
            t_negdiag = persist.tile([128, 128], bf16, tag="ndg")
            nc.sync.dma_start(t_ident[:], ident[:])
            nc.sync.dma_start(t_negdiag[:], negdiag[:])
            t_ones = persist.tile([128, 128], bf16, tag="ones")
            nc.vector.memset(t_ones[:], 1.0)
            t_bias4 = persist.tile([128, 1], f32, tag="bias4")
            t_bias0 = persist.tile([128, 1], f32, tag="bias0")
            nc.vector.memset(t_bias4[:], R2)
            nc.vector.memset(t_bias0[:], 0.0)
            t_ones6 = persist.tile([1, 8], f32, tag="ones6")
            nc.vector.memset(t_ones6[:], 1.0)

            # ---- persistent per-cloud operand tiles ----
            A5 = {}; W5 = {}; F10 = {}; FT = {}
            for cl in ("p", "g"):
                A5[cl] = persist.tile([5, N], f32, tag=f"A5{cl}", name=f"A5{cl}")
                W5[cl] = persist.tile([5, N], f32, tag=f"W5{cl}", name=f"W5{cl}")
                F10[cl] = persist.tile([10, N], f32, tag=f"F10{cl}", name=f"F10{cl}")
                FT[cl] = persist.tile([128, NB * 20], bf16, tag=f"FT{cl}",
                                      name=f"FT{cl}")

            t_rowmax = persist.tile([128, NB, 2], f32, tag="rowmax")
            t_s1 = persist.tile([128, NB], f32, tag="s1")
            t_s2 = persist.tile([128, NB], f32, tag="s2")

            # ================= on-device prep =================
            # A5 = [2x, 2y, 2z, nn, 1] (fp32 matmul lhs rows)
            # W5 = [x, y, z, -1, -nn]  (fp32 matmul rhs rows)
            # F10 = [x2,xy,xz,y2,yz,z2,x,y,z,1] of centered coords
            # FT  = transposed bf16 hi/lo features [128, kb*20 + (0:10 hi|10:20 lo)]
            def prep(src_dram, cl):
                P3 = scrp.tile([3, N], f32, tag="P3")
                nc.sync.dma_start(P3[:], src_dram[:].rearrange("a b -> b a"))
                S3 = scrp.tile([3, N], f32, tag="S3")
                nc.vector.tensor_tensor(S3[:], P3[:], P3[:], Alu.mult)
                r1 = scrp.tile([1, N], f32, tag="r1")
                r2t = scrp.tile([1, N], f32, tag="r2t")
                nc.sync.dma_start(r1[:], S3[1:2, :])
                nc.sync.dma_start(r2t[:], S3[2:3, :])
                nn = scrp.tile([1, N], f32, tag="nn")
                nc.vector.tensor_tensor(nn[:], S3[0:1, :], r1[:], Alu.add)
                nc.vector.tensor_tensor(nn[:], nn[:], r2t[:], Alu.add)
                # engine ops may only start at partitions {0,32,64,96}: memset
                # the whole tile for the constant rows, DMA the odd-row writes
                a5, w5 = A5[cl], W5[cl]
                nc.vector.memset(a5[:], 1.0)
                nc.scalar.activation(a5[0:3, :], P3[:], Act.Copy, scale=2.0)
                nc.sync.dma_start(a5[3:4, :], nn[:])
                nc.vector.memset(w5[:], -1.0)
                nc.vector.tensor_copy(w5[0:3, :], P3[:])
                nnn = scrp.tile([1, N], f32, tag="nnn")
                nc.scalar.activation(nnn[:], nn[:], Act.Copy, scale=-1.0)
                nc.sync.dma_start(w5[4:5, :], nnn[:])
                # centered features
                C3 = scrp.tile([3, N], f32, tag="C3")
                nc.vector.tensor_scalar_add(C3[:], P3[:], -0.5)
                A6 = scrp.tile([6, N], f32, tag="A6")
                B6 = scrp.tile([6, N], f32, tag="B6")
                # A6 rows = [c0,c0,c0,c1,c1,c2]; B6 rows = [c0,c1,c2,c1,c2,c2]
                nc.vector.tensor_copy(A6[0:1, :], C3[0:1, :])
                nc.sync.dma_start(A6[1:2, :], C3[0:1, :])
                nc.sync.dma_start(A6[2:3, :], C3[0:1, :])
                nc.sync.dma_start(A6[3:4, :], C3[1:2, :])
                nc.sync.dma_start(A6[4:5, :], C3[1:2, :])
                nc.sync.dma_start(A6[5:6, :], C3[2:3, :])
                nc.vector.tensor_copy(B6[0:3, :], C3[:])
                nc.sync.dma_start(B6[3:5, :], C3[1:3, :])
                nc.sync.dma_start(B6[5:6, :], C3[2:3, :])
                f10 = F10[cl]
                nc.vector.memset(f10[:], 1.0)
                nc.vector.tensor_tensor(f10[0:6, :], A6[:], B6[:], Alu.mult)
                nc.sync.dma_start(f10[6:9, :], C3[:])
                # bf16 hi/lo split of features
                hi10 = scrp.tile([10, N], bf16, tag="hi10")
                hif = scrp.tile([10, N], f32, tag="hif")
                lo10f = scrp.tile([10, N], f32, tag="lo10f")
                lo10 = scrp.tile([10, N], bf16, tag="lo10")
                nc.scalar.activation(hi10[:], f10[:], Act.Copy)
                nc.scalar.activation(hif[:], hi10[:], Act.Copy)
                nc.vector.tensor_tensor(lo10f[:], f10[:], hif[:], Alu.subtract)
                nc.scalar.activation(lo10[:], lo10f[:], Act.Copy)
                # transpose [10, 128]-chunks -> FT[:, kb*20 + 0:10 / 10:20]
                ftt = FT[cl]
                for b in range(NB):
                    csl = slice(b * 128, (b + 1) * 128)
                    pst = psd.tile([128, 16], bf16, tag="dps")
                    nc.tensor.transpose(pst[:, 0:10], hi10[:, csl],
                                        t_ident[0:10, 0:10])
                    nc.scalar.activation(ftt[:, b * 20:b * 20 + 10], pst[:, 0:10],
                                         Act.Copy)
                    pst2 = psd.tile([128, 16], bf16, tag="dps")
                    nc.tensor.transpose(pst2[:, 0:10], lo10[:, csl],
                                        t_ident[0:10, 0:10])
                    nc.scalar.activation(ftt[:, b * 20 + 10:b * 20 + 20],
                                         pst2[:, 0:10], Act.Copy)

            prep(pred, "p")
            prep(gt, "g")

            # fp32 distance matmul: psum[128, 1024] = -D block (row block b,
            # column half h) between clouds (a5 lhs, w5 rhs)
            def build_half(a5, w5, b, h, ps):
                for j in range(2):
                    nc.tensor.matmul(
                        ps[:, j * 512:(j + 1) * 512],
                        a5[:, b * 128:(b + 1) * 128],
                        w5[:, h * 1024 + j * 512:h * 1024 + (j + 1) * 512],
                        start=True, stop=True,
                    )

            # ================= phase 1: chamfer on -Dpg =================
            t_colacc = big.tile([128, N], f32, tag="bigA")
            t_colred = big.tile([128, N], f32, tag="bigB")
            for b in range(NB):
                for h in range(2):
                    ps = psd.tile([128, 1024], f32, tag="dps")
                    build_half(A5["p"], W5["g"], b, h, ps)
                    nc.vector.tensor_reduce(t_rowmax[:, b, h:h + 1],
                                            ps[:], Axis.X, Alu.max)
                    cslice = slice(h * 1024, (h + 1) * 1024)
                    if b == 0:
                        nc.vector.tensor_copy(t_colacc[:, cslice], ps[:])
                    else:
                        nc.vector.tensor_tensor(t_colacc[:, cslice],
                                                t_colacc[:, cslice], ps[:], Alu.max)
            # partition-tree max 128 -> 1 (DMA crosses partitions, DVE cannot)
            for h in [64, 32, 16, 8, 4, 2, 1]:
                nc.sync.dma_start(t_colred[0:h, :], t_colacc[h:2 * h, :])
                nc.vector.tensor_tensor(t_colacc[0:h, :], t_colacc[0:h, :],
                                        t_colred[0:h, :], Alu.max)
            # chamfer scalars: sum of per-row maxes + sum of col maxes (of -D)
            t_cdcol = persist.tile([1, 1], f32, tag="cdcol")
            nc.vector.tensor_reduce(t_cdcol[:], t_colacc[0:1, :], Axis.X, Alu.add)
            t_rowfull = scrp.tile([128, NB], f32, tag="rowfull")
            nc.vector.tensor_reduce(t_rowfull[:], t_rowmax[:], Axis.X, Alu.max)
            t_cdrow = persist.tile([128, 1], f32, tag="cdrow")
            nc.vector.tensor_reduce(t_cdrow[:], t_rowfull[:], Axis.X, Alu.add)

            # ================= phases 2-4 for pp and gg =================
            def normals_phase(cl, out_dram, do_rep):
                a5, w5, ftt, f10 = A5[cl], W5[cl], FT[cl], F10[cl]

                def build_ndm(b):
                    ndm = ndmp.tile([128, N], bf16, tag="ndm", name=f"ndm{cl}{b}")
                    for h in range(2):
                        ps = psd.tile([128, 1024], f32, tag="dps")
                        build_half(a5, w5, b, h, ps)
                        nc.scalar.activation(ndm[:, h * 1024:(h + 1) * 1024],
                                             ps[:], Act.Copy)
                    nc.vector.tensor_tensor(
                        ndm[:, b * 128:(b + 1) * 128],
                        ndm[:, b * 128:(b + 1) * 128],
                        t_negdiag[:], Alu.add)
                    return ndm

                # pass 1: repulsion moments + 16-NN radius (tau) per row
                t_tau = scrp.tile([128, NB], f32, tag="tau")
                for b in range(NB):
                    ndm = build_ndm(b)
                    if do_rep:
                        scr = scrp.tile([128, N], bf16, tag="repscr")
                        scr2 = scrp.tile([128, N], bf16, tag="repscr2")
                        nc.scalar.activation(scr[:], ndm[:], Act.Relu,
                                             bias=t_bias4[:],
                                             accum_out=t_s1[:, b:b + 1])
                        nc.scalar.activation(scr2[:], scr[:], Act.Square,
                                             bias=t_bias0[:],
                                             accum_out=t_s2[:, b:b + 1])
                    t1 = scrp.tile([128, 1024], bf16, tag="tree1")
                    At = scrp.tile([128, 512], bf16, tag="treeA")
                    At2 = scrp.tile([128, 512], bf16, tag="treeA2")
                    m8a = scrp.tile([128, 8], bf16, tag="m8a")
                    m8b = scrp.tile([128, 8], bf16, tag="m8b")
                    nc.vector.tensor_tensor(t1[:], ndm[:, 0:1024],
                                            ndm[:, 1024:2048], Alu.max)
                    nc.vector.tensor_tensor(At[:], t1[:, 0:512],
                                            t1[:, 512:1024], Alu.max)
                    nc.vector.max(m8a[:], At[:])
                    nc.vector.match_replace(At2[:], m8a[:], At[:], float(NEG_BIG))
                    nc.vector.max(m8b[:], At2[:])
                    nc.vector.tensor_copy(t_tau[:, b:b + 1], m8b[:, 6:7])

                # tau broadcast: per-row tau -> [1, N] row -> PE ones-matmul
                # broadcast across partitions; mask compare is then direct on
                # the SYMMETRIC ndm blocks: wt[j, i] = (ndm[j, i] >= tau_i)
                t_taub = scrp.tile([128, 128], bf16, tag="taub")
                nc.vector.memset(t_taub[:], 0.0)
                nc.vector.tensor_copy(t_taub[:, 0:NB], t_tau[:])
                ps_tt = psd.tile([128, 128], bf16, tag="dps")
                nc.tensor.transpose(ps_tt[:], t_taub[:], t_ident[:])
                t_tt = scrp.tile([NB, 128], bf16, tag="tts")
                nc.scalar.activation(t_tt[:], ps_tt[0:NB, :], Act.Copy)
                t_tauT = scrp.tile([128, N], bf16, tag="tauT")
                nc.vector.memset(t_tauT[:], 0.0)
                nc.sync.dma_start(t_tauT[0:1, :], t_tt[:])
                t_taubc = scrp.tile([128, N], bf16, tag="taubc")
                for h in range(2):
                    ps_tau = psd.tile([128, 1024], f32, tag="dps")
                    for bb in range(8):
                        c0 = h * 1024 + bb * 128
                        nc.tensor.matmul(ps_tau[:, bb * 128:(bb + 1) * 128],
                                         t_ones[:],
                                         t_tauT[:, c0:c0 + 128],
                                         start=True, stop=True)
                    nc.scalar.activation(t_taubc[:, h * 1024:(h + 1) * 1024],
                                         ps_tau[:], Act.Copy)

                # pass 2: rebuild -D per block, mask, accumulate covariance
                # moments cps[10, N] over kb (hi+lo)
                cps = psc.tile([10, N], f32, tag="cps")
                for kb in range(NB):
                    ndm = build_ndm(kb)
                    wt = wtp.tile([128, N], bf16, tag="wt", name=f"wt{cl}{kb}")
                    nc.vector.tensor_tensor(wt[:], ndm[:], t_taubc[:], Alu.is_ge)
                    for j in range(4):
                        cols = slice(j * 512, (j + 1) * 512)
                        for half in range(2):
                            nc.tensor.matmul(
                                cps[:, cols],
                                ftt[:, kb * 20 + half * 10:kb * 20 + (half + 1) * 10],
                                wt[:, cols],
                                start=(kb == 0 and half == 0),
                                stop=(kb == NB - 1 and half == 1))
                # self add, then center on device:
                #   covc[ab] = M2[ab]/cnt - (s[a]/cnt)*(s[b]/cnt)   (f16 out)
                covsb = big.tile([10, N], f32, tag="bigA", name=f"covsb{cl}")
                nc.vector.tensor_tensor(covsb[:], cps[:], f10[:], Alu.add)
                rr = scrp.tile([1, N], f32, tag="r1")
                nc.sync.dma_start(rr[:], covsb[9:10, :])
                rcp = scrp.tile([1, N], f32, tag="r2t")
                nc.vector.reciprocal(rcp[:], rr[:])
                mus = scrp.tile([3, N], f32, tag="S3")
                nc.sync.dma_start(mus[:], covsb[6:9, :])
                psB3 = psc.tile([3, N], f32, tag="cps", name=f"psB3{cl}")
                for j in range(4):
                    cj = slice(j * 512, (j + 1) * 512)
                    nc.tensor.matmul(psB3[:, cj], t_ones6[0:1, 0:3], rcp[:, cj],
                                     start=True, stop=True)
                mu3 = scrp.tile([3, N], f32, tag="C3")
                nc.vector.tensor_tensor(mu3[:], mus[:], psB3[:], Alu.mult)
                A6m = scrp.tile([6, N], f32, tag="A6")
                B6m = scrp.tile([6, N], f32, tag="B6")
                nc.vector.tensor_copy(A6m[0:1, :], mu3[0:1, :])
                nc.sync.dma_start(A6m[1:2, :], mu3[0:1, :])
                nc.sync.dma_start(A6m[2:3, :], mu3[0:1, :])
                nc.sync.dma_start(A6m[3:4, :], mu3[1:2, :])
                nc.sync.dma_start(A6m[4:5, :], mu3[1:2, :])
                nc.sync.dma_start(A6m[5:6, :], mu3[2:3, :])
                nc.vector.tensor_copy(B6m[0:3, :], mu3[:])
                nc.sync.dma_start(B6m[3:5, :], mu3[1:3, :])
                nc.sync.dma_start(B6m[5:6, :], mu3[2:3, :])
                P6 = scrp.tile([6, N], f32, tag="lo10f")
                nc.vector.tensor_tensor(P6[:], A6m[:], B6m[:], Alu.mult)
                psB6 = psc.tile([6, N], f32, tag="cps", name=f"psB6{cl}")
                for j in range(4):
                    cj = slice(j * 512, (j + 1) * 512)
                    nc.tensor.matmul(psB6[:, cj], t_ones6[0:1, 0:6], rcp[:, cj],
                                     start=True, stop=True)
                M2r = scrp.tile([6, N], f32, tag="hif")
                nc.vector.tensor_tensor(M2r[:], covsb[0:6, :], psB6[:], Alu.mult)
                covc = scrp.tile([6, N], f16, tag="hi10")
                nc.vector.tensor_tensor(covc[:], M2r[:], P6[:], Alu.subtract)
                nc.sync.dma_start(out_dram[:, 0:N], covc[:])

            normals_phase("p", out_p, do_rep=True)

            # ---- repulsion moment inversion -> per-row contribution ----
            # a,b = (s1 +- sqrt(2*s2 - s1^2))/2; d=sqrt(r2-v); contrib =
            # relu(0.02-da)+relu(0.02-db), gated by s1>0
            sh = [128, NB]
            t_t1 = scrp.tile(sh, f32, tag="rp1")
            t_t2 = scrp.tile(sh, f32, tag="rp2")
            t_sq = scrp.tile(sh, f32, tag="rp3")
            t_va = scrp.tile(sh, f32, tag="rp4")
            t_vb = scrp.tile(sh, f32, tag="rp5")
            t_ca = scrp.tile(sh, f32, tag="rp6")
            t_cb = scrp.tile(sh, f32, tag="rp7")
            t_msk = scrp.tile(sh, f32, tag="rp8")
            Alu_ = Alu
            nc.vector.tensor_tensor(t_t1[:], t_s1[:], t_s1[:], Alu_.mult)
            nc.vector.tensor_scalar(t_t2[:], t_s2[:], 2.0, None, Alu_.mult)
            nc.vector.tensor_tensor(t_t2[:], t_t2[:], t_t1[:], Alu_.subtract)
            nc.vector.tensor_scalar_max(t_t2[:], t_t2[:], 0.0)
            nc.scalar.activation(t_sq[:], t_t2[:], Act.Sqrt)
            nc.vector.tensor_tensor(t_va[:], t_s1[:], t_sq[:], Alu_.add)
            nc.vector.tensor_scalar(t_va[:], t_va[:], 0.5, R2, Alu_.mult, Alu_.min)
            nc.vector.tensor_tensor(t_vb[:], t_s1[:], t_sq[:], Alu_.subtract)
            nc.vector.tensor_scalar(t_vb[:], t_vb[:], 0.5, 0.0, Alu_.mult, Alu_.max)
            # da = sqrt(max(r2 - va, 1e-12)); contrib_a = max(0.02 - da, 0)
            for tv, tc_ in ((t_va, t_ca), (t_vb, t_cb)):
                nc.vector.tensor_scalar(tv[:], tv[:], -1.0, R2, Alu_.mult, Alu_.add)
                nc.vector.tensor_scalar_max(tv[:], tv[:], 1e-12)
                nc.scalar.activation(tv[:], tv[:], Act.Sqrt)
                nc.vector.tensor_scalar(tc_[:], tv[:], -1.0, float(REP_THRESH),
                                        Alu_.mult, Alu_.add)
                nc.vector.tensor_scalar_max(tc_[:], tc_[:], 0.0)
            nc.vector.tensor_scalar(t_msk[:], t_s1[:], 0.0, None, Alu_.is_gt)
            nc.vector.tensor_tensor(t_ca[:], t_ca[:], t_cb[:], Alu_.add)
            nc.vector.tensor_tensor(t_ca[:], t_ca[:], t_msk[:], Alu_.mult)
            t_reprow = persist.tile([128, 1], f32, tag="reprow")
            nc.vector.tensor_reduce(t_reprow[:], t_ca[:], Axis.X, Alu_.add)

            # ---- partition-sum [cd_row, rep] via DMA tree; pack scalars ----
            t_P2 = scrp.tile([128, 2], f32, tag="P2")
            t_P2s = scrp.tile([64, 2], f32, tag="P2s")
            nc.vector.tensor_copy(t_P2[:, 0:1], t_cdrow[:])
            nc.vector.tensor_copy(t_P2[:, 1:2], t_reprow[:])
            for h in [64, 32, 16, 8, 4, 2, 1]:
                nc.sync.dma_start(t_P2s[0:h, :], t_P2[h:2 * h, :])
                nc.vector.tensor_tensor(t_P2[0:h, :], t_P2[0:h, :],
                                        t_P2s[0:h, :], Alu.add)
            t_z6 = scrp.tile([6, 32], f16, tag="z6")
            nc.vector.memset(t_z6[:], 0.0)
            nc.vector.tensor_copy(t_z6[0:1, 0:2], t_P2[0:1, :])
            nc.vector.tensor_copy(t_z6[0:1, 2:3], t_cdcol[:])
            nc.sync.dma_start(out_g[:, N:N + 32], t_z6[:])

            normals_phase("g", out_g, do_rep=False)

    if split_waits:
        _split_excess_waits(nc, mybir)
    return nc


def _split_excess_waits(nc, mybir, max_w=1, max_u=1):
    """This toolchain's walrus accepts at most 1 sync wait and 1 update per
    instruction. Move excess waits onto same-engine prefix NoOps (the engine
    is in-order, so waiting earlier is equivalent) and excess updates onto
    suffix NoOps (signalling marginally later is safe)."""
    n = 0
    for func in nc.m.functions:
        for block in func.blocks:
            lst = block.instructions
            new = []
            for inst in lst:
                si = inst.sync_info
                ow = list(si.on_wait) if (si and si.on_wait) else []
                if len(ow) > max_w:
                    extra, keep = ow[:-max_w], ow[-max_w:]
                    for k in range(0, len(extra), max_w):
                        nop = mybir.InstNoOp(name=f"I-wsplit-{n}"); n += 1
                        nop.engine = inst.engine
                        nop.sync_info = mybir.SyncInfo(
                            on_wait=extra[k:k + max_w], on_update=[])
                        new.append(nop)
                    si.on_wait = keep
                new.append(inst)
                ou = list(si.on_update) if (si and si.on_update) else []
                if len(ou) > max_u:
                    keep_u, extra_u = ou[:max_u], ou[max_u:]
                    si.on_update = keep_u
                    for k in range(0, len(extra_u), max_u):
                        nop = mybir.InstNoOp(name=f"I-usplit-{n}"); n += 1
                        nop.engine = inst.engine
                        nop.sync_info = mybir.SyncInfo(
                            on_wait=[], on_update=extra_u[k:k + max_u])
                        new.append(nop)
            lst[:] = new
    return n


_NC_CACHE = None


def _get_nc():
    global _NC_CACHE
    if _NC_CACHE is None:
        _NC_CACHE = _build_nc()
    return _NC_CACHE


def _consts_np():
    negdiag = np.zeros((128, 128), dtype=BF16)
    np.fill_diagonal(negdiag, BF16(NEG_BIG))
    ident = np.zeros((128, 128), dtype=BF16)
    np.fill_diagonal(ident, BF16(1.0))
    return ident, negdiag


# ============================================================================
# Cached jit runner (replicates bass2jax.run_bass_via_pjrt, but the jitted
# executable, mesh, and const device buffers are built ONCE; the donated
# output buffer is recycled from the previous call's output)
# ============================================================================

class _Runner:
    def __init__(self):
        import jax
        from jax.sharding import Mesh, PartitionSpec, NamedSharding
        from jax.experimental.shard_map import shard_map
        from concourse import bass2jax
        import concourse.mybir as mybir

        self.jax = jax
        nc = _get_nc()
        bass2jax.install_neuronx_cc_hook()

        partition_name = (nc.partition_id_tensor.name
                          if nc.partition_id_tensor else None)
        in_names, out_names, out_avals, zero_outs = [], [], [], []
        for alloc in nc.m.functions[0].allocations:
            if not isinstance(alloc, mybir.MemoryLocationSet):
                continue
            name = alloc.memorylocations[0].name
            if alloc.kind == "ExternalInput":
                if name != partition_name:
                    in_names.append(name)
            elif alloc.kind == "ExternalOutput":
                shape = tuple(alloc.tensor_shape)
                dtype = mybir.dt.np(alloc.dtype)
                out_names.append(name)
                out_avals.append(jax.core.ShapedArray(shape, dtype))
                zero_outs.append((shape, dtype))
        assert in_names == ["pred", "gt", "ident", "negdiag"], in_names
        assert out_names == ["out_p", "out_g"], out_names
        n_params = len(in_names)
        n_outs = len(out_names)
        all_names = in_names + out_names
        if partition_name is not None:
            all_names.append(partition_name)
        self.zero_outs = zero_outs

        def _body(*args):
            operands = list(args)
            if partition_name is not None:
                operands.append(bass2jax.partition_id_tensor())
            outs = bass2jax._bass_exec_p.bind(
                *operands,
                out_avals=tuple(out_avals),
                in_names=tuple(all_names),
                out_names=tuple(out_names),
                lowering_input_output_aliases=(),
                sim_require_finite=True,
                sim_require_nnan=True,
                nc=nc,
            )
            return tuple(outs)

        devices = jax.devices()[:B]
        assert len(devices) == B, f"need {B} devices, have {len(jax.devices())}"
        mesh = Mesh(np.asarray(devices), ("core",))
        pspec = PartitionSpec("core")
        self._fn = jax.jit(
            shard_map(_body, mesh=mesh,
                      in_specs=(pspec,) * (n_params + n_outs),
                      out_specs=(pspec,) * n_outs,
                      check_rep=False),
            donate_argnums=tuple(range(n_params, n_params + n_outs)),
            keep_unused=True,
        )
        ident, negdiag = _consts_np()
        sh = NamedSharding(mesh, pspec)
        self._ident = jax.device_put(np.tile(ident, (B, 1)), sh)
        self._negdiag = jax.device_put(np.tile(negdiag, (B, 1)), sh)
        self._donate = None  # recycled output buffers

    def run(self, pred, gt):
        """pred, gt: [B, N, 3] f32 -> (fut_p, fut_g) resolving to host
        np.ndarrays [B*6, N] / [B*6, N+32] f16."""
        zeros = self._donate
        if zeros is None:
            zeros = [np.zeros((B * s[0],) + s[1:], d)
                     for s, d in self.zero_outs]
        out_p, out_g = self._fn(pred.reshape(B * N, DIM),
                                gt.reshape(B * N, DIM),
                                self._ident, self._negdiag, *zeros)
        # queue both host transfers EAGERLY: an np.asarray issued after the
        # ready notification pays a full extra tunnel round trip (~100ms);
        # copy_to_host_async rides the execute pipeline instead, and cloud
        # g's transfer proceeds in background while the host runs cloud p's
        # eigensolve
        try:
            out_p.copy_to_host_async()
            out_g.copy_to_host_async()
        except Exception:
            pass
        # the kernel writes every element of both outputs, so last call's
        # outputs can be donated as the next call's output buffers
        self._donate = [out_p, out_g]
        return out_p, out_g


_RUNNER = None


def _get_runner():
    global _RUNNER
    if _RUNNER is None:
        _RUNNER = _Runner()
    return _RUNNER


# ============================================================================
# Host combine
# ============================================================================

# ----------------------------------------------------------------------------
# LAPACK ssyevd 3x3 sign-convention replication (fp32), numba scalar port of
# the vectorized replica validated 100% against jax/scipy CPU eigh signs.
# Falls back to np.linalg.eigh (99.35% sign agreement) without numba.
# ----------------------------------------------------------------------------
try:
    from numba import njit as _njit
    _HAVE_NUMBA = True
except Exception:  # pragma: no cover
    _HAVE_NUMBA = False

if _HAVE_NUMBA:
    _F = np.float32
    _EPS = _F(2.0) ** _F(-24)
    _EPS2 = _F(_EPS * _EPS)
    _SAFMIN = _F(1.1754943508222875e-38)
    _ONE = _F(1.0)
    _TWO = _F(2.0)
    _HALF = _F(0.5)
    _ZERO = _F(0.0)

    @_njit(cache=True, fastmath=False)
    def _fsign(a, b):
        return np.abs(a) if b >= _ZERO else -np.abs(a)

    @_njit(cache=True, fastmath=False)
    def _slapy2(x, y):
        ax = np.abs(x); ay = np.abs(y)
        w = max(ax, ay); z = min(ax, ay)
        if z == _ZERO:
            return w
        r = z / w
        return w * np.sqrt(_ONE + r * r)

    @_njit(cache=True, fastmath=False)
    def _slartg(f, g):
        if g == _ZERO:
            return _ONE, _ZERO, f
        if f == _ZERO:
            return _ZERO, _fsign(_ONE, g), np.abs(g)
        d = np.sqrt(f * f + g * g)
        cs = np.abs(f) / d
        r = _fsign(d, f)
        sn = g / r
        return cs, sn, r

    @_njit(cache=True, fastmath=False)
    def _slaev2(a, b, c):
        sm = a + c
        df = a - c
        adf = np.abs(df)
        tb = b + b
        ab_ = np.abs(tb)
        if np.abs(a) > np.abs(c):
            acmx = a; acmn = c
        else:
            acmx = c; acmn = a
        if adf > ab_:
            r_ = ab_ / adf
            rt = adf * np.sqrt(_ONE + r_ * r_)
        elif adf < ab_:
            r_ = adf / ab_
            rt = ab_ * np.sqrt(_ONE + r_ * r_)
        else:
            rt = ab_ * np.sqrt(_TWO)
        if sm < _ZERO:
            rt1 = _HALF * (sm - rt)
            sgn1 = -_ONE
            rt2 = (acmx / rt1) * acmn - (b / rt1) * b
        elif sm > _ZERO:
            rt1 = _HALF * (sm + rt)
            sgn1 = _ONE
            rt2 = (acmx / rt1) * acmn - (b / rt1) * b
        else:
            rt1 = _HALF * rt
            sgn1 = _ONE
            rt2 = -_HALF * rt
        if df >= _ZERO:
            cs = df + rt
            sgn2 = _ONE
        else:
            cs = df - rt
            sgn2 = -_ONE
        acs = np.abs(cs)
        if acs > ab_:
            ct = -tb / cs
            sn1 = _ONE / np.sqrt(_ONE + ct * ct)
            cs1 = ct * sn1
        else:
            if ab_ == _ZERO:
                cs1 = _ONE
                sn1 = _ZERO
            else:
                tn = -cs / tb
                cs1 = _ONE / np.sqrt(_ONE + tn * tn)
                sn1 = tn * cs1
        if sgn1 == sgn2:
            t = cs1
            cs1 = -sn1
            sn1 = t
        return rt1, rt2, cs1, sn1

    @_njit(cache=True, fastmath=False)
    def _rot(Z, ca, cb, c, s):
        for i in range(3):
            temp = Z[i, cb]
            Z[i, cb] = c * temp - s * Z[i, ca]
            Z[i, ca] = s * temp + c * Z[i, ca]

    @_njit(cache=True, fastmath=False)
    def _eigh3_batch(cv, out):
        # cv: [Bc, 6, N] f32 rows (xx, xy, xz, yy, yz, zz); out: [Bc*N, 3]
        Z = np.empty((3, 3), np.float32)
        n_pts = cv.shape[2]
        for idx in range(cv.shape[0] * n_pts):
            bb = idx // n_pts
            nn_ = idx - bb * n_pts
            a00 = cv[bb, 0, nn_]; a10 = cv[bb, 1, nn_]; a20 = cv[bb, 2, nn_]
            a11 = cv[bb, 3, nn_]; a21 = cv[bb, 4, nn_]; a22 = cv[bb, 5, nn_]
            # ssytd2 lower
            xnorm = np.abs(a20)
            alpha = a10
            beta = -_fsign(_slapy2(alpha, xnorm), alpha)
            refl = xnorm != _ZERO
            if refl:
                tau1 = (beta - alpha) / beta
                v2 = a20 / (alpha - beta)
                w1 = tau1 * a11 + tau1 * (a21 * v2)
                w2 = tau1 * a21 + (tau1 * v2) * a22
                alp = -_HALF * tau1 * (w1 + w2 * v2)
                w1 = w1 + alp
                w2 = w2 + alp * v2
                d0 = a00
                d1 = a11 - (w1 + w1)
                d2 = a22 - ((v2 * w2) + (v2 * w2))
                e0 = beta
                e1 = a21 - (v2 * w1 + w2)
            else:
                tau1 = _ZERO
                v2 = _ZERO
                d0 = a00; d1 = a11; d2 = a22
                e0 = a10; e1 = a21
            for i in range(3):
                for j in range(3):
                    Z[i, j] = _ONE if i == j else _ZERO
            s0 = np.abs(e0) <= (np.sqrt(np.abs(d0)) * np.sqrt(np.abs(d1))) * _EPS
            s1m = np.abs(e1) <= (np.sqrt(np.abs(d1)) * np.sqrt(np.abs(d2))) * _EPS
            if s0:
                e0 = _ZERO
            if s1m:
                e1 = _ZERO
            if s0 and not s1m:
                tst = e1 * e1
                thr = (_EPS2 * np.abs(d1)) * np.abs(d2) + _SAFMIN
                if tst > thr:
                    rt1, rt2, c, s = _slaev2(d1, e1, d2)
                    _rot(Z, 1, 2, c, s)
                    d1 = rt1; d2 = rt2
                e1 = _ZERO
            elif (not s0) and s1m:
                tst = e0 * e0
                thr = (_EPS2 * np.abs(d0)) * np.abs(d1) + _SAFMIN
                if tst > thr:
                    rt1, rt2, c, s = _slaev2(d0, e0, d1)
                    _rot(Z, 0, 1, c, s)
                    d0 = rt1; d1 = rt2
                e0 = _ZERO
            elif (not s0) and (not s1m):
                if np.abs(d2) < np.abs(d0):
                    # QR variant
                    l = 2
                    for _it in range(40):
                        if l <= -1:
                            break
                        if l == 2:
                            m2s = e1 * e1 <= (_EPS2 * np.abs(d2)) * np.abs(d1) + _SAFMIN
                            m1s = e0 * e0 <= (_EPS2 * np.abs(d1)) * np.abs(d0) + _SAFMIN
                            if m2s:
                                e1 = _ZERO
                                l = 1
                            elif m1s:
                                e0 = _ZERO
                                rt1, rt2, c, s = _slaev2(d1, e1, d2)
                                _rot(Z, 1, 2, c, s)
                                d1 = rt1; d2 = rt2
                                e1 = _ZERO
                                l = 0
                            else:
                                P = d2
                                G = (d1 - P) / (_TWO * e1)
                                R = _slapy2(G, _ONE)
                                G = d0 - P + (e1 / (G + _fsign(R, G)))
                                Fv = e0
                                Bv = e0
                                C, S, R = _slartg(G, Fv)
                                G2 = d0
                                R = (d1 - G2) * S + (_TWO * C) * Bv
                                Pv = S * R
                                d0n = G2 + Pv
                                G = C * R - Bv
                                c0 = C; s0_ = S
                                Fv = S * e1
                                Bv = C * e1
                                C, S, R = _slartg(G, Fv)
                                e0n = R
                                G2 = d1 - Pv
                                R = (d2 - G2) * S + (_TWO * C) * Bv
                                Pv2 = S * R
                                d1n = G2 + Pv2
                                G = C * R - Bv
                                c1 = C; s1_ = S
                                _rot(Z, 0, 1, c0, s0_)
                                _rot(Z, 1, 2, c1, s1_)
                                d0 = d0n; d1 = d1n; d2 = d2 - Pv2
                                e0 = e0n; e1 = G
                        elif l == 1:
                            ms = e0 * e0 <= (_EPS2 * np.abs(d1)) * np.abs(d0) + _SAFMIN
                            if ms:
                                e0 = _ZERO
                                l = 0
                            else:
                                rt1, rt2, c, s = _slaev2(d0, e0, d1)
                                _rot(Z, 0, 1, c, s)
                                d0 = rt1; d1 = rt2
                                e0 = _ZERO
                                l = -1
                        else:  # l == 0
                            l = -1
                else:
                    # QL variant
                    l = 0
                    for _it in range(40):
                        if l >= 3:
                            break
                        if l == 0:
                            m0s = e0 * e0 <= (_EPS2 * np.abs(d0)) * np.abs(d1) + _SAFMIN
                            m1s = e1 * e1 <= (_EPS2 * np.abs(d1)) * np.abs(d2) + _SAFMIN
                            if m0s:
                                e0 = _ZERO
                                l = 1
                            elif m1s:
                                e1 = _ZERO
                                rt1, rt2, c, s = _slaev2(d0, e0, d1)
                                _rot(Z, 0, 1, c, s)
                                d0 = rt1; d1 = rt2
                                e0 = _ZERO
                                l = 2
                            else:
                                P = d0
                                G = (d1 - P) / (_TWO * e0)
                                R = _slapy2(G, _ONE)
                                G = d2 - P + (e0 / (G + _fsign(R, G)))
                                Fv = e1
                                Bv = e1
                                C, S, R = _slartg(G, Fv)
                                G2 = d2
                                R = (d1 - G2) * S + (_TWO * C) * Bv
                                Pv = S * R
                                d2n = G2 + Pv
                                G = C * R - Bv
                                c1 = C; s1_ = -S
                                Fv = S * e0
                                Bv = C * e0
                                C, S, R = _slartg(G, Fv)
                                e1n = R
                                G2 = d1 - Pv
                                R = (d0 - G2) * S + (_TWO * C) * Bv
                                Pv2 = S * R
                                d1n = G2 + Pv2
                                G = C * R - Bv
                                c0 = C; s0_ = -S
                                _rot(Z, 1, 2, c1, s1_)
                                _rot(Z, 0, 1, c0, s0_)
                                d2 = d2n; d1 = d1n; d0 = d0 - Pv2
                                e1 = e1n; e0 = G
                        elif l == 1:
                            ms = e1 * e1 <= (_EPS2 * np.abs(d1)) * np.abs(d2) + _SAFMIN
                            if ms:
                                e1 = _ZERO
                                l = 2
                            else:
                                rt1, rt2, c, s = _slaev2(d1, e1, d2)
                                _rot(Z, 1, 2, c, s)
                                d1 = rt1; d2 = rt2
                                e1 = _ZERO
                                l = 3
                        else:  # l == 2
                            l = 3
            # sort eigenvalues ascending, swapping Z columns (ssteqr tail)
            D0 = d0; D1 = d1; D2 = d2
            for i in range(2):
                if i == 0:
                    k = 0; P = D0
                    if D1 < P:
                        k = 1; P = D1
                    if D2 < P:
                        k = 2; P = D2
                    if k != 0:
                        if k == 1:
                            D1 = D0
                        else:
                            D2 = D0
                        D0 = P
                        for r_i in range(3):
                            t = Z[r_i, 0]; Z[r_i, 0] = Z[r_i, k]; Z[r_i, k] = t
                else:
                    if D2 < D1:
                        t2 = D1; D1 = D2; D2 = t2
                        for r_i in range(3):
                            t = Z[r_i, 1]; Z[r_i, 1] = Z[r_i, 2]; Z[r_i, 2] = t
            # back-transform the householder (sorm2r)
            if refl:
                for col in range(3):
                    w = Z[1, col] + v2 * Z[2, col]
                    Z[1, col] = Z[1, col] - tau1 * w
                    Z[2, col] = Z[2, col] - (tau1 * v2) * w
            out[idx, 0] = Z[0, 0]
            out[idx, 1] = Z[1, 0]
            out[idx, 2] = Z[2, 0]


def _normals_from_covc(cv):
    """cv: [B, 6, N] centered covariance rows [xx,xy,xz,yy,yz,zz] (f16) ->
    [B*N, 3] smallest-eigval eigenvectors with ssyevd sign convention."""
    f32 = np.float32
    cv32 = cv.astype(f32)
    if _HAVE_NUMBA:
        out = np.empty((cv32.shape[0] * cv32.shape[2], 3), f32)
        _eigh3_batch(cv32, out)
        return out
    flat = np.ascontiguousarray(cv32.transpose(0, 2, 1).reshape(-1, 6))
    cov = np.empty((flat.shape[0], 3, 3), dtype=f32)
    cov[:, 0, 0] = flat[:, 0]
    cov[:, 0, 1] = cov[:, 1, 0] = flat[:, 1]
    cov[:, 0, 2] = cov[:, 2, 0] = flat[:, 2]
    cov[:, 1, 1] = flat[:, 3]
    cov[:, 1, 2] = cov[:, 2, 1] = flat[:, 4]
    cov[:, 2, 2] = flat[:, 5]
    return np.linalg.eigh(cov)[1][:, :, 0]


def _host_combine(out_p, out_g):
    """out_p/out_g: device outputs [B*6, N] / [B*6, N+32] f16 (transfers
    already queued via copy_to_host_async) -> scalar loss f32."""
    arr_p = np.asarray(out_p).reshape(B, 6, N)
    n_p = _normals_from_covc(arr_p)  # overlaps cloud-g transfer
    arr_g = np.asarray(out_g).reshape(B, 6, N + 32)
    n_g = _normals_from_covc(arr_g[:, :, 0:N])
    dots = (n_p * n_g).sum(-1)
    normc = 1.0 - dots.mean(dtype=np.float64)

    scal = arr_g[:, 0, N:N + 3].astype(np.float64)
    cd = -(scal[:, 0].sum() + scal[:, 2].sum()) / (B * N)
    rep = scal[:, 1].sum() / (B * N * K_REP)

    return np.float32(CD_W * cd + REP_W * rep + NORM_W * normc)


# ============================================================================
# Entry point
# ============================================================================

def kernel(pred, gt):
    pred = np.ascontiguousarray(np.asarray(pred, dtype=np.float32))
    gt = np.ascontiguousarray(np.asarray(gt, dtype=np.float32))
    assert pred.shape == (B, N, DIM) and gt.shape == (B, N, DIM)
    out_p, out_g = _get_runner().run(pred, gt)
    return _host_combine(out_p, out_g)


if __name__ == "__main__":
    rng = np.random.default_rng(0)
    pred = rng.uniform(size=(B, N, DIM)).astype(np.float32)
    gt = rng.uniform(size=(B, N, DIM)).astype(np.float32)
    print("loss:", kernel(pred, gt))
